# revision 12
# baseline (speedup 1.0000x reference)
"""Trainium2 Bass kernel for an enhanced transformer layer.

Strategy: data-parallel over batch (B=8 -> one batch element per NeuronCore,
no collectives).  Activations are kept feature-major ([D, S] with the
contraction dim on partitions) so every linear layer consumes weights in
natural [K, E] layout as the stationary operand.  Matmuls run in bf16 with
fp32 PSUM accumulation.

Schedule: the layer is software-pipelined over the two S/2 column chunks
(qc) so LayerNorm / softmax vector work overlaps matmul streams:
  v-proj (staged x arrival) -> qk-proj -> attn(qc0) -> [attn(hp, qc1) |
  Wo(qc0)] -> lna(qc0) | Wo(qc1).a -> n1(qc0) | Wo(qc1).b ->
  FFN-W1(qc0) | lna(qc1) -> FFN-W2(qc0) | n1(qc1) -> FFN-W3/gate(qc0) with
  incremental n2(qc0) stats -> n2(qc0) | FFN(qc1) -> n2(qc1).
LayerNorm statistics use M=128 ones-matmuls that produce row-broadcast sums
directly (no scalar-row chains, no separate broadcast matmuls), and
1/sqrt(var) is computed on DVE via Newton iteration so the ACT engine never
loads the sqrt table (a table-set switch costs ~2.7us).  Softmax
denominators are likewise accumulated as broadcast ones-matmuls; exp is
evaluated on paired PSUM banks ([128, 2*SQ] per instruction) to halve ACT
dispatch count in the attention phase.  The depthwise-conv residual and
h=x+attn adds run on GPSIMD (Pool).  The attention residual accumulator
(conv + Wo output, without x) is kept in bf16; x is added back in fp32 at
LN time.  All small constants ship in one packed DMA (dma_start dispatch
costs ~0.65us each on the sync engine).
"""

import math

import numpy as np
import ml_dtypes

import concourse.bass as bass
import concourse.tile as tile
from concourse import mybir
from concourse.alu_op_type import AluOpType
from bass_rust import ScopedClock

F32 = mybir.dt.float32
I32 = mybir.dt.int32
BF16 = mybir.dt.bfloat16
F16 = mybir.dt.float16
AF = mybir.ActivationFunctionType
OP = AluOpType

EPS = 1e-5
N_CORES = 8
MAGIC = 0x5F3759DF


class CFG:
    def __init__(self, S=1024, D=1024, F=4096, H=16):
        self.S, self.D, self.F, self.H = S, D, F, H
        self.DK = D // H              # head dim (must be 64)
        self.KD = D // 128            # feature tiles of model dim
        self.KF = F // 128            # feature tiles of ffn dim
        self.SQ = min(512, S)         # moving-dim chunk
        self.NQ = S // self.SQ
        self.NKT = S // 128           # key/sequence tiles
        self.VC = min(512, D)         # v-projection output chunk
        self.NVC = D // self.VC
        assert self.DK == 64 and H % 2 == 0


FULL = CFG()


def _split_excess_waits(nc, max_waits=1):
    """Walrus in this container rejects >2 sync waits per instruction.
    Hoist excess waits onto same-engine nops inserted just before."""
    cnt = 0
    for fn in nc.m.functions:
        for bb in fn.blocks:
            insts = list(bb.instructions)
            out = []
            for inst in insts:
                si = inst.sync_info
                waits = list(si.on_wait) if si and si.on_wait else []
                if len(waits) > max_waits:
                    extra = waits[:-max_waits]
                    si.on_wait = waits[-max_waits:]
                    for i in range(0, len(extra), max_waits):
                        cnt += 1
                        out.append(mybir.InstNoOp(
                            name=f"waitsplit{cnt}_{inst.name}",
                            engine=inst.engine, ins=[], outs=[],
                            sync_info=mybir.SyncInfo(
                                on_wait=extra[i:i + max_waits], on_update=[]),
                        ))
                out.append(inst)
            if cnt:
                bb.instructions = out
    return cnt


class _TC(tile.TileContext):
    """TileContext whose exit drain spreads semaphore waits over several
    sync-engine nops -- this container's walrus rejects >2 sync waits on a
    single CTRL instruction."""

    def __exit__(self, *a):
        r = super().__exit__(*a)
        _split_excess_waits(self.nc)
        return r

    def _drain_and_barrier(self, tick_clock, wait_clock):
        nc = self.nc
        drain_inst = nc.sync.drain()
        wait_clock.add_sem_waits(
            drain_inst.ins, ScopedClock({None: tick_clock.global_clock})
        )
        si = drain_inst.ins.sync_info
        waits = list(si.on_wait) if si and si.on_wait else []
        if len(waits) > 1:
            si.on_wait = waits[:1]
            for w in waits[1:]:
                nop = nc.sync.nop(nofuse=True)
                nsi = nop.ins.sync_info
                if nsi is None:
                    nop.ins.sync_info = mybir.SyncInfo(on_wait=[w], on_update=[])
                else:
                    nsi.on_wait = [w]
        nc.all_engine_barrier()
        popped = nc._tile_sem_poison_stack.pop()
        assert popped is self._sem_poison
        nc.clear_and_free_semaphores(list(self.sems.allocated().values()))
        nc.all_engine_barrier()


def emit(tc, cfg, io):
    nc = tc.nc
    S, D, F, H = cfg.S, cfg.D, cfg.F, cfg.H
    DK, KD, KF = cfg.DK, cfg.KD, cfg.KF
    SQ, NQ, NKT = cfg.SQ, cfg.NQ, cfg.NKT
    VC, NVC = cfg.VC, cfg.NVC
    HP = H // 2

    def qs(qc):
        return slice(qc * SQ, (qc + 1) * SQ)

    pool = tc.alloc_tile_pool

    # ======== pools: alloc order is EXACT reverse death order per side ====
    consts = pool(name="consts", bufs=1)
    tmpp = pool(name="tmp", bufs=1)
    vecp = pool(name="vec", bufs=1)
    sqp = pool(name="sq", bufs=1)
    smallp = pool(name="small", bufs=1)
    stgp = pool(name="stg", bufs=1)
    wd = pool(name="wd", bufs=1)
    rp = pool(name="racc", bufs=1)           # bf16 conv+Wo residual
    xpp = pool(name="xp", bufs=1)
    xbfp = pool(name="xbf", bufs=1)          # x bf16, then reused as h
    vtp = pool(name="vt", bufs=1, side="right")
    wvp = pool(name="wv", bufs=1, side="right")
    xhp = pool(name="xh", bufs=1, side="right")  # fp16 x staging
    aop = pool(name="aop", bufs=1)
    qkp = pool(name="qk", bufs=1)
    expp = pool(name="expT", bufs=1)
    psum = pool(name="ps", bufs=1, space="PSUM")

    # ---------------- constants (one packed DMA) ----------------
    cpk = consts.tile([128, 16 * KD + 2 * KF], F32, name="cpk", tag="cpk")
    ct = {}
    off = 0
    for cname in ("bq", "bk", "bocb", "cw0", "cw1", "cw2", "b3t", "bgt",
                  "lnag", "lnab", "n1g", "n1b", "n2g", "n2b", "bdla", "bdn1"):
        ct[cname] = cpk[:, off:off + KD]
        off += KD
    for cname in ("b1t", "b2t"):
        ct[cname] = cpk[:, off:off + KF]
        off += KF
    ones_bf = consts.tile([128, 128], BF16, name="onesbf", tag="onesbf")
    nc.vector.memset(ones_bf[:], 1.0)
    bvr_bf = consts.tile([1, D], BF16, name="bvrbf", tag="bvrbf")
    nc.sync.dma_start(bvr_bf[:], io["bvr"][:, :])
    nc.sync.dma_start(cpk[:], io["cpk"][:, :])

    def ps_tile():
        return psum.tile([128, SQ], F32, name="ps", tag="ps", bufs=2)

    def pp_tile():
        return psum.tile([128, 2, SQ], F32, name="pp", tag="pp", bufs=3)

    # ---------------- x + wv loads, staged for earliest v-proj ------------
    xp, xbf = [], []
    for kt in range(KD):
        t = xpp.tile([128, S + 2], F32, name=f"xp{kt}", tag=f"xp{kt}")
        xp.append(t)
        b = xbfp.tile([128, S], BF16, name=f"xbf{kt}", tag=f"xbf{kt}")
        xbf.append(b)
    wv = [wvp.tile([128, D], BF16, name=f"wv{kt}", tag=f"wv{kt}")
          for kt in range(KD)]
    for kt in range(KD):
        nc.sync.dma_start(wv[kt][:], io["wv"][kt * 128:(kt + 1) * 128, :])
    # x arrives fp16; stage through a rotating SBUF tile, then fan out to
    # the bf16 matmul copy (DVE) and the fp32 conv/LN copy (DVE).
    def stage_x(kt, c0, c1):
        xh = xhp.tile([128, 512], F16, name="xh", tag="xh",
                      bufs=3)[:, :c1 - c0]
        nc.sync.dma_start(xh, io["xT"][kt * 128:(kt + 1) * 128, c0:c1])
        nc.vector.tensor_copy(xbf[kt][:, c0:c1], xh)
        nc.vector.tensor_copy(xp[kt][:, 1 + c0:1 + c1], xh)

    # first seq tile of x (enables v-proj st=0), then the rest
    for kt in range(KD):
        stage_x(kt, 0, 128)
    for kt in range(KD):
        stage_x(kt, 128, SQ)
    for kt in range(KD):
        stage_x(kt, SQ, S)
        nc.vector.memset(xp[kt][:, 0:1], 0.0)
        nc.vector.memset(xp[kt][:, S + 1:S + 2], 0.0)

    # ---------------- v projection (x stationary, wv moving) --------------
    v_t = [vtp.tile([128, D], BF16, name=f"vt{st}", tag=f"vt{st}")
           for st in range(NKT)]
    for st in range(NKT):
        for ec in range(NVC):
            ps = ps_tile()
            for kt in range(KD):
                nc.tensor.matmul(ps[:, 0:VC],
                                 xbf[kt][:, st * 128:(st + 1) * 128],
                                 wv[kt][:, ec * VC:(ec + 1) * VC],
                                 start=(kt == 0), stop=False)
            # bv bias via K=1 ones-row matmul into the same accumulation
            nc.tensor.matmul(ps[:, 0:VC], ones_bf[0:1, 0:128],
                             bvr_bf[0:1, ec * VC:(ec + 1) * VC],
                             start=False, stop=True)
            nc.vector.tensor_copy(v_t[st][:, ec * VC:(ec + 1) * VC],
                                  ps[:, 0:VC])
    xhp.release()
    wvp.release()

    # ---------------- conv residual on GPSIMD: racc = 0.3*conv ------------
    # (x itself is NOT accumulated here; it is added back in fp32 at LN
    #  time.  racc is bf16: it only ever holds conv + attention output.)
    racc = [rp.tile([128, S], BF16, name=f"ra{kt}", tag=f"ra{kt}")
            for kt in range(KD)]
    for kt in range(KD):
        for qc in range(NQ):
            o = qc * SQ
            xl, xc, xr = (xp[kt][:, o:o + SQ], xp[kt][:, o + 1:o + SQ + 1],
                          xp[kt][:, o + 2:o + SQ + 2])
            t1 = tmpp.tile([128, SQ], F32, name="lnt", tag="lnt", bufs=4)
            nc.vector.tensor_scalar(t1[:], xl, ct["cw0"][:, kt:kt + 1], None,
                                    op0=OP.mult)
            t2 = tmpp.tile([128, SQ], F32, name="lnt", tag="lnt", bufs=4)
            nc.vector.scalar_tensor_tensor(t2[:], xc, ct["cw1"][:, kt:kt + 1],
                                           t1[:], op0=OP.mult, op1=OP.add)
            nc.vector.scalar_tensor_tensor(racc[kt][:, qs(qc)], xr,
                                           ct["cw2"][:, kt:kt + 1], t2[:],
                                           op0=OP.mult, op1=OP.add)

    # ---------------- q/k projection (paired psum: both qc at once) -------
    q_fm = [qkp.tile([128, S], BF16, name=f"q{m}", tag=f"q{m}")
            for m in range(KD)]
    k_fm = [qkp.tile([128, S], BF16, name=f"k{m}", tag=f"k{m}")
            for m in range(KD)]

    def wd_tile():
        return wd.tile([128, D], BF16, name="wd", tag="wd", bufs=3)

    def qk_proj(m):
        for wname, bias, dst in (("wqs", ct["bq"], q_fm),
                                 ("wks", ct["bk"], k_fm)):
            wt = wd_tile()
            nc.sync.dma_start(wt[:], io[wname][:, m * D:(m + 1) * D])
            pp = pp_tile()
            for qc in range(NQ):
                for kt in range(KD):
                    nc.tensor.matmul(pp[:, qc, :],
                                     wt[:, kt * 128:(kt + 1) * 128],
                                     xbf[kt][:, qs(qc)],
                                     start=(kt == 0), stop=(kt == KD - 1))
            nc.scalar.activation(dst[m][:], pp[:], AF.Identity,
                                 bias=bias[:, m:m + 1])

    # ---------------- attention ----------------
    inv_sqrt_dk = 1.0 / math.sqrt(DK)
    _ao = []

    def attn(hp, qc):
        attnout = _ao
        h0, h1 = 2 * hp, 2 * hp + 1
        eps_ = []
        for kt in range(NKT):
            pp = pp_tile()
            nc.tensor.matmul(pp[:, 0, :],
                             k_fm[hp][0:64, kt * 128:(kt + 1) * 128],
                             q_fm[hp][0:64, qs(qc)], start=True, stop=True)
            nc.tensor.matmul(pp[:, 1, :],
                             k_fm[hp][64:128, kt * 128:(kt + 1) * 128],
                             q_fm[hp][64:128, qs(qc)], start=True, stop=True)
            ep = expp.tile([128, 2, SQ], BF16, name="exp", tag="exp", bufs=9)
            nc.scalar.activation(ep[:], pp[:], AF.Exp, scale=inv_sqrt_dk)
            eps_.append(ep)
        den_t, U_t = ps_tile(), ps_tile()
        den, U = den_t[:, :], U_t[:, :]
        for kt in range(NKT):
            nc.tensor.matmul(den[0:64, :], ones_bf[:, 0:64], eps_[kt][:, 0, :],
                             start=(kt == 0), stop=(kt == NKT - 1))
        for kt in range(NKT):
            nc.tensor.matmul(den[64:128, :], ones_bf[:, 0:64],
                             eps_[kt][:, 1, :],
                             start=(kt == 0), stop=(kt == NKT - 1))
        for kt in range(NKT):
            nc.tensor.matmul(U[0:64, :], v_t[kt][:, h0 * DK:(h0 + 1) * DK],
                             eps_[kt][:, 0, :],
                             start=(kt == 0), stop=(kt == NKT - 1))
        for kt in range(NKT):
            nc.tensor.matmul(U[64:128, :], v_t[kt][:, h1 * DK:(h1 + 1) * DK],
                             eps_[kt][:, 1, :],
                             start=(kt == 0), stop=(kt == NKT - 1))
        recb = smallp.tile([128, SQ], F32, name="recb", tag="recb", bufs=2)
        nc.vector.reciprocal(recb[:], den)
        nc.vector.tensor_tensor(attnout[hp][:, qs(qc)], U, recb[:],
                                op=OP.mult)

    # ---------------- Wo accumulate into racc ----------------
    def wo_step(m, qc):
        attnout = _ao
        wt = wd_tile()
        nc.sync.dma_start(wt[:], io["wos"][:, m * D:(m + 1) * D])
        ps = ps_tile()
        for kt in range(KD):
            nc.tensor.matmul(ps[:], wt[:, kt * 128:(kt + 1) * 128],
                             attnout[kt][:, qs(qc)],
                             start=(kt == 0), stop=(kt == KD - 1))
        nc.vector.scalar_tensor_tensor(racc[m][:, qs(qc)], ps[:],
                                       ct["bocb"][:, m:m + 1],
                                       racc[m][:, qs(qc)],
                                       op0=OP.add, op1=OP.add)

    # ---------------- layernorm helpers (broadcast stats) ----------------
    def vtile(tag, dt=F32):
        return vecp.tile([128, SQ], dt, name=tag, tag=tag, bufs=1)

    def ln_stats(cast_fn):
        """Returns psum pair (ssum at [:,0,:], ssq at [:,1,:])."""
        sp = pp_tile()
        for kt in range(KD):
            rbf = cast_fn(kt)
            sq_t = sqp.tile([128, SQ], BF16, name="sq", tag="sq", bufs=2)
            nc.vector.tensor_tensor(sq_t[:], rbf[:], rbf[:], op=OP.mult)
            st_, sp_ = (kt == 0), (kt == KD - 1)
            nc.tensor.matmul(sp[:, 0, :], ones_bf[:, 0:128], rbf[:],
                             start=st_, stop=sp_)
            nc.tensor.matmul(sp[:, 1, :], ones_bf[:, 0:128], sq_t[:],
                             start=st_, stop=sp_)
        return sp

    def ln_finish(sp, c0=0, c1=None):
        """mu, rstd [128, c1-c0] from the stats pair (Newton rsqrt on DVE)."""
        c1 = SQ if c1 is None else c1
        mu_t, var_t, rst_t = vtile("vmu"), vtile("vvar"), vtile("vrst")
        mu, var, rst = mu_t[:, c0:c1], var_t[:, c0:c1], rst_t[:, c0:c1]
        nc.vector.tensor_scalar_mul(mu, sp[:, 0, c0:c1], 1.0 / D)
        nc.vector.tensor_tensor(var, mu, mu, op=OP.mult)
        nc.vector.scalar_tensor_tensor(var, sp[:, 1, c0:c1], 1.0 / D, var,
                                       op0=OP.mult, op1=OP.subtract)
        nc.vector.tensor_scalar_add(var, var, EPS)
        rst_i = rst_t[:].bitcast(I32)[:, c0:c1]
        var_i = var_t[:].bitcast(I32)[:, c0:c1]
        nc.vector.tensor_scalar(rst_i, var_i, 1, None,
                                op0=OP.arith_shift_right)
        nc.vector.tensor_scalar(rst_i, rst_i, -1, MAGIC, op0=OP.mult,
                                op1=OP.add)
        for _ in range(2):
            t = vtile("vnt")[:, c0:c1]
            nc.vector.tensor_tensor(t, rst, rst, op=OP.mult)
            nc.vector.tensor_tensor(t, t, var, op=OP.mult)
            nc.vector.tensor_scalar(t, t, -0.5, 1.5, op0=OP.mult,
                                    op1=OP.add)
            nc.vector.tensor_tensor(rst, rst, t, op=OP.mult)
        return mu, rst

    def ln_norm_simple(src_fn, mu, rst, write_out, c0=0, c1=None):
        c1 = SQ if c1 is None else c1
        for kt in range(KD):
            t1 = tmpp.tile([128, SQ], F32, name="lnt", tag="lnt", bufs=4)
            nc.vector.tensor_tensor(t1[:, c0:c1], src_fn(kt), mu,
                                    op=OP.subtract)
            nc.vector.tensor_tensor(t1[:, c0:c1], t1[:, c0:c1], rst,
                                    op=OP.mult)
            write_out(kt, t1)

    # ---------------- LN chains ----------------
    def lna_r2(qc):
        """lna on (x + racc); then r2 = x + lna_out accumulated into xp."""
        def cast_lna(kt):
            rbf = sqp.tile([128, SQ], BF16, name="rbf", tag="rbf", bufs=2)
            nc.gpsimd.tensor_tensor(
                rbf[:], xp[kt][:, 1 + qc * SQ:1 + (qc + 1) * SQ],
                racc[kt][:, qs(qc)], op=OP.add)
            return rbf

        sp = ln_stats(cast_lna)
        mu, rst = ln_finish(sp)
        for kt in range(KD):
            xs = xp[kt][:, 1 + qc * SQ:1 + (qc + 1) * SQ]
            t1 = tmpp.tile([128, SQ], F32, name="lnt", tag="lnt", bufs=4)
            nc.vector.tensor_tensor(t1[:], xs, racc[kt][:, qs(qc)], op=OP.add)
            nc.vector.tensor_tensor(t1[:], t1[:], mu[:], op=OP.subtract)
            nc.vector.tensor_tensor(t1[:], t1[:], rst[:], op=OP.mult)
            t2 = tmpp.tile([128, SQ], F32, name="lnt", tag="lnt", bufs=4)
            nc.scalar.activation(t2[:], t1[:], AF.Identity,
                                 bias=ct["lnab"][:, kt:kt + 1],
                                 scale=ct["lnag"][:, kt:kt + 1])
            nc.gpsimd.tensor_tensor(xs, xs, t2[:], op=OP.add)

    h_holder = []

    def n1_chain(qc):
        h_bf = h_holder

        def cast_n1(kt):
            rbf = sqp.tile([128, SQ], BF16, name="rbf", tag="rbf", bufs=2)
            nc.gpsimd.tensor_copy(rbf[:],
                                  xp[kt][:, 1 + qc * SQ:1 + (qc + 1) * SQ])
            return rbf

        sp = ln_stats(cast_n1)
        mu, rst = ln_finish(sp)

        def w_n1(kt, t1):
            nc.scalar.activation(h_bf[kt][:, qs(qc)], t1[:], AF.Identity,
                                 bias=ct["n1b"][:, kt:kt + 1],
                                 scale=ct["n1g"][:, kt:kt + 1])

        ln_norm_simple(
            lambda kt: xp[kt][:, 1 + qc * SQ:1 + (qc + 1) * SQ], mu, rst, w_n1)

    # ========================================================================
    # schedule
    # ========================================================================
    for m in range(KD):
        qk_proj(m)
    _ao.extend(aop.tile([128, S], BF16, name=f"ao{m}", tag=f"ao{m}")
               for m in range(KD))
    h_holder.extend(xbf)    # reuse the dead x-bf16 tiles as h storage
    for hp in range(HP):
        attn(hp, 0)
    for m in range(KD):
        wo_step(m, 0)
    lna_r2(0)               # chains run on DVE/Pool under attn(qc1) below
    for hp in range(HP // 2):
        attn(hp, 1)
    n1_chain(0)
    for hp in range(HP // 2, HP):
        attn(hp, 1)
    expp.release()
    qkp.release()
    vtp.release()

    for m in range(KD):
        wo_step(m, 1)
    aop.release()
    lna_r2(1)               # chain overlaps FFN-W1(qc0)

    f1p = pool(name="ffn1", bufs=1, side="right")

    def ffn_w1(qc, mid=None):
        f1t = []
        for m in range(KF):
            if m == 8 and mid is not None:
                mid()
            wt = wd_tile()
            nc.sync.dma_start(wt[:], io["w1s"][:, m * D:(m + 1) * D])
            ps = ps_tile()
            for kt in range(KD):
                nc.tensor.matmul(ps[:], wt[:, kt * 128:(kt + 1) * 128],
                                 h_holder[kt][:, qs(qc)],
                                 start=(kt == 0), stop=(kt == KD - 1))
            t = f1p.tile([128, SQ], BF16, name=f"f1_{m}", tag=f"f1_{m}")
            nc.scalar.activation(t[:], ps[:], AF.Gelu,
                                 bias=ct["b1t"][:, m:m + 1])
            f1t.append(t)
        return f1t

    f1_qc0 = ffn_w1(0, mid=lambda: n1_chain(1))

    wf = pool(name="wf", bufs=1)
    fop = pool(name="fout", bufs=1)
    f2p = pool(name="ffn2", bufs=1, side="right")
    fout = [fop.tile([128, SQ], F32, name=f"fo{m}", tag=f"fo{m}")
            for m in range(KD)]

    def wf_tile():
        """Half of a W2/W3 contraction block: [128, F//2]."""
        return wf.tile([128, F // 2], BF16, name="wf", tag="wf", bufs=3)

    def big_mm(ps, wname, m, rhs_tiles):
        """Accumulate over KF k-tiles streaming weights in two half tiles."""
        for h in range(2):
            wt = wf_tile()
            nc.sync.dma_start(wt[:], io[wname][:, m * F + h * (F // 2):
                                               m * F + (h + 1) * (F // 2)])
            for j in range(KF // 2):
                kt = h * (KF // 2) + j
                nc.tensor.matmul(ps, wt[:, j * 128:(j + 1) * 128],
                                 rhs_tiles[kt][:],
                                 start=(kt == 0), stop=(kt == KF - 1))

    def ffn_rest(qc, f1t, mid=None):
        h_bf = h_holder
        f2t = []
        for m in range(KF):
            if m == 8 and mid is not None:
                mid()
            ps = ps_tile()
            big_mm(ps[:], "w2s", m, f1t)
            t = f2p.tile([128, SQ], BF16, name=f"f2_{m}", tag=f"f2_{m}")
            nc.scalar.activation(t[:], ps[:], AF.Gelu,
                                 bias=ct["b2t"][:, m:m + 1])
            f2t.append(t)
        # W3 + gate, with incremental n2 statistics per output tile
        n2sp = pp_tile()
        for m in range(KD):
            wtg = wd_tile()
            nc.sync.dma_start(wtg[:], io["wgs"][:, m * D:(m + 1) * D])
            gp = pp_tile()
            psg, ps3 = gp[:, 0, :], gp[:, 1, :]
            for kt in range(KD):
                nc.tensor.matmul(psg, wtg[:, kt * 128:(kt + 1) * 128],
                                 h_bf[kt][:, qs(qc)],
                                 start=(kt == 0), stop=(kt == KD - 1))
            gat = tmpp.tile([128, SQ], BF16, name="gat", tag="gat", bufs=2)
            nc.scalar.activation(gat[:], psg, AF.Sigmoid,
                                 bias=ct["bgt"][:, m:m + 1])
            big_mm(ps3, "w3s", m, f2t)
            t = tmpp.tile([128, SQ], F32, name="f3t", tag="f3t", bufs=2)
            nc.vector.scalar_tensor_tensor(t[:], ps3, ct["b3t"][:, m:m + 1],
                                           gat[:], op0=OP.add, op1=OP.mult)
            nc.vector.tensor_tensor(fout[m][:], t[:],
                                    h_bf[m][:, qs(qc)], op=OP.add)
            # incremental n2 stats for this feature tile
            rbf = sqp.tile([128, SQ], BF16, name="rbf", tag="rbf", bufs=2)
            nc.gpsimd.tensor_copy(rbf[:], fout[m][:])
            sq_t = sqp.tile([128, SQ], BF16, name="sq", tag="sq", bufs=2)
            nc.scalar.square(sq_t[:], rbf[:])
            st_, sp_ = (m == 0), (m == KD - 1)
            nc.tensor.matmul(n2sp[:, 0, :], ones_bf[:, 0:128], rbf[:],
                             start=st_, stop=sp_)
            nc.tensor.matmul(n2sp[:, 1, :], ones_bf[:, 0:128], sq_t[:],
                             start=st_, stop=sp_)
        return n2sp

    def emit_n2(qc, n2sp):
        mu, rst = ln_finish(n2sp)

        def w_n2(kt, t1):
            stg = stgp.tile([128, SQ], F16, name="stg", tag="stg", bufs=2)
            nc.scalar.activation(stg[:], t1[:], AF.Identity,
                                 bias=ct["n2b"][:, kt:kt + 1],
                                 scale=ct["n2g"][:, kt:kt + 1])
            nc.sync.dma_start(io["outT"][kt * 128:(kt + 1) * 128, qs(qc)],
                              stg[:])

        ln_norm_simple(lambda kt: fout[kt][:], mu, rst, w_n2)

    n2sp0 = ffn_rest(0, f1_qc0)
    emit_n2(0, n2sp0)
    f1_qc1 = ffn_w1(1)
    n2sp1 = ffn_rest(1, f1_qc1)
    emit_n2(1, n2sp1)

    f2p.release()
    f1p.release()
    fop.release()
    wf.release()
    xbfp.release()
    xpp.release()
    rp.release()
    wd.release()
    stgp.release()
    smallp.release()
    sqp.release()
    vecp.release()
    tmpp.release()
    consts.release()
    psum.release()


# ------------------------------------------------------------------
# host side
# ------------------------------------------------------------------

def _shuffle_w(w):
    """[K, E] -> [128, (E//128)*K] bf16 so that slice [:, m*K:(m+1)*K]
    viewed as [128, K//128, 128] gives lhsT tiles w[kt*128+p, m*128+c]."""
    K, E = w.shape
    r = np.asarray(w).reshape(K // 128, 128, E // 128, 128).transpose(1, 2, 0, 3)
    return np.ascontiguousarray(r.reshape(128, (E // 128) * K)).astype(
        ml_dtypes.bfloat16)


def _ptable(b):
    """[E] -> [128, E//128] per-partition scalar table."""
    return np.ascontiguousarray(np.asarray(b, np.float32).reshape(-1, 128).T)


def _declare_io(nc, cfg, consts=None):
    """xT/outT are per-call I/O; every weight tensor is embedded in the
    NEFF as a Const DRAM tensor (DMA'd to HBM once at model-load time) so
    repeated executions only ship the activation tensors."""
    S, D, F, KD, KF = cfg.S, cfg.D, cfg.F, cfg.KD, cfg.KF
    io = {}

    def inp(name, shape, dt):
        if consts is not None and name in consts:
            arr = np.ascontiguousarray(consts[name])
            assert list(arr.shape) == list(shape), (name, arr.shape, shape)
            io[name] = nc.inline_tensor(arr, name=name).ap()
        else:
            io[name] = nc.dram_tensor(name, shape, dt, kind="ExternalInput").ap()

    inp("xT", [D, S], F16)
    inp("wqs", [128, KD * D], BF16)
    inp("wks", [128, KD * D], BF16)
    inp("wv", [D, D], BF16)
    inp("wos", [128, KD * D], BF16)
    inp("w1s", [128, KF * D], BF16)
    inp("w2s", [128, KF * F], BF16)
    inp("wgs", [128, KD * D], BF16)
    inp("w3s", [128, KD * F], BF16)
    inp("cpk", [128, 16 * KD + 2 * KF], F32)
    inp("bvr", [1, D], BF16)
    io["outT"] = nc.dram_tensor("outT", [D, S], F16, kind="ExternalOutput").ap()
    return io


def build_shared_inputs(inputs, cfg):
    """Everything except xT (identical across cores)."""
    f32 = np.float32
    g = {k: np.asarray(v) for k, v in inputs.items()}
    ptabs = [
        _ptable(g["bq"]), _ptable(g["bk"]),
        _ptable(np.asarray(g["bo"], f32) + 0.3 * np.asarray(g["conv_b"], f32)),
        _ptable(0.3 * np.asarray(g["conv_w"], f32)[:, 0]),
        _ptable(0.3 * np.asarray(g["conv_w"], f32)[:, 1]),
        _ptable(0.3 * np.asarray(g["conv_w"], f32)[:, 2]),
        _ptable(g["b3"]), _ptable(g["bg"]),
        _ptable(g["lna_g"]), _ptable(g["lna_b"]),
        _ptable(g["n1_g"]), _ptable(g["n1_b"]),
        _ptable(g["n2_g"]), _ptable(g["n2_b"]),
        _ptable(np.asarray(g["lna_b"], f32)
                / np.where(np.abs(np.asarray(g["lna_g"], f32)) < 1e-20,
                           1.0, np.asarray(g["lna_g"], f32))),
        _ptable(np.asarray(g["n1_b"], f32)
                / np.where(np.abs(np.asarray(g["n1_g"], f32)) < 1e-20,
                           1.0, np.asarray(g["n1_g"], f32))),
        _ptable(g["b1"]), _ptable(g["b2"]),
    ]
    sh = {
        "wqs": _shuffle_w(g["Wq"]), "wks": _shuffle_w(g["Wk"]),
        "wv": np.ascontiguousarray(g["Wv"]).astype(ml_dtypes.bfloat16),
        "wos": _shuffle_w(g["Wo"]), "w1s": _shuffle_w(g["W1"]),
        "w2s": _shuffle_w(g["W2"]), "w3s": _shuffle_w(g["W3"]),
        "wgs": _shuffle_w(g["Wg"]),
        "cpk": np.ascontiguousarray(np.concatenate(ptabs, axis=1)),
        "bvr": np.ascontiguousarray(
            np.asarray(g["bv"], f32).reshape(1, cfg.D)).astype(
                ml_dtypes.bfloat16),
    }
    return sh


_CACHE = {}


def _weights_fingerprint(inputs):
    """Content hash of every non-x input. The NEFF embeds the weights, so
    a changed weight set must rebuild (and recompile) the kernel."""
    import hashlib
    h = hashlib.blake2b(digest_size=16)
    for k in sorted(inputs):
        if k == "x":
            continue
        a = np.ascontiguousarray(inputs[k])
        h.update(k.encode())
        h.update(str(a.shape).encode())
        h.update(str(a.dtype).encode())
        h.update(a.tobytes())
    return h.hexdigest()


def _get_nc(inputs):
    key = _weights_fingerprint(inputs)
    ent = _CACHE.get("nc")
    if ent is None or ent[0] != key:
        shared = build_shared_inputs(inputs, FULL)
        nc = bass.Bass("TRN2", target_bir_lowering=False, debug=False)
        io = _declare_io(nc, FULL, consts=shared)
        with _TC(nc) as tc:
            emit(tc, FULL, io)
        _CACHE["nc"] = (key, nc)
    return _CACHE["nc"][1]


def _get_exec(inputs):
    """Persistent jitted executor: one bass_exec custom call shard_mapped
    over the 8 cores.  Rebuilt only when the weight set changes."""
    import jax
    from jax.sharding import Mesh, PartitionSpec
    from jax.experimental.shard_map import shard_map
    from concourse import bass2jax

    key = _weights_fingerprint(inputs)
    ent = _CACHE.get("exec")
    if ent is not None and ent[0] == key:
        return ent[1]

    bass2jax.install_neuronx_cc_hook()
    nc = _get_nc(inputs)
    pid = nc.partition_id_tensor.name if nc.partition_id_tensor else None
    in_names, out_names, out_avals = [], [], []
    for alloc in nc.m.functions[0].allocations:
        if not isinstance(alloc, mybir.MemoryLocationSet):
            continue
        name = alloc.memorylocations[0].name
        if alloc.kind == "ExternalInput":
            if name != pid:
                in_names.append(name)
        elif alloc.kind == "ExternalOutput":
            out_names.append(name)
            out_avals.append(jax.core.ShapedArray(
                tuple(alloc.tensor_shape), mybir.dt.np(alloc.dtype)))
    all_names = in_names + ([pid] if pid is not None else [])
    assert in_names == ["xT"] and out_names == ["outT"]

    def _body(*args):
        operands = list(args)
        if pid is not None:
            operands.append(bass2jax.partition_id_tensor())
        return tuple(bass2jax._bass_exec_p.bind(
            *operands, out_avals=tuple(out_avals), in_names=tuple(all_names),
            out_names=tuple(out_names), lowering_input_output_aliases=(),
            sim_require_finite=True, sim_require_nnan=True, nc=nc))

    mesh = Mesh(np.asarray(jax.devices()[:N_CORES]), ("core",))
    sharded = jax.jit(
        shard_map(_body, mesh=mesh, in_specs=(PartitionSpec("core"),),
                  out_specs=(PartitionSpec("core"),), check_rep=False),
        keep_unused=True)
    _CACHE["exec"] = (key, sharded)
    return sharded


def kernel(**inputs):
    import jax

    cfg = FULL
    sharded = _get_exec(inputs)
    x = np.asarray(inputs["x"], dtype=np.float32)
    B = x.shape[0]
    assert B == N_CORES
    xc = np.concatenate([x[b].T.astype(np.float16, order="C")
                         for b in range(B)], axis=0)
    out = sharded(jax.device_put(xc))
    o0 = np.asarray(out[0]).astype(np.float32).reshape(B, cfg.D, cfg.S)
    return np.ascontiguousarray(o0.transpose(0, 2, 1))



# revision 13
# speedup vs baseline: 1.0194x; 1.0194x over previous
"""Trainium2 Bass kernel for an enhanced transformer layer.

Strategy: data-parallel over batch (B=8 -> one batch element per NeuronCore,
no collectives).  Activations are kept feature-major ([D, S] with the
contraction dim on partitions) so every linear layer consumes weights in
natural [K, E] layout as the stationary operand.  Matmuls run in bf16 with
fp32 PSUM accumulation.

I/O strategy: all weight tensors are embedded in the NEFF as Const DRAM
tensors (DMA'd to HBM once at model-load time), so per-execution bindings
are just xT (fp16 in) and outT (fp16 out) -- 2 MB each per core.  The NEFF
is rebuilt (keyed on a content hash of the weights) if kernel() is called
with a different weight set.  fp16 transport adds ~2^-11 relative
quantization noise on x and out, negligible against the bf16 matmul noise.

Schedule: the layer is software-pipelined over the two S/2 column chunks
(qc) so LayerNorm / softmax vector work overlaps matmul streams:
  v-proj (staged x arrival) -> qk-proj -> attn(qc0) -> [attn(hp, qc1) |
  Wo(qc0)] -> lna(qc0) | Wo(qc1).a -> n1(qc0) | Wo(qc1).b ->
  FFN-W1(qc0) | lna(qc1) -> FFN-W2(qc0) | n1(qc1) -> FFN-W3/gate(qc0) with
  incremental n2(qc0) stats -> n2(qc0) | FFN(qc1) -> n2(qc1).
LayerNorm statistics use M=128 ones-matmuls that produce row-broadcast sums
directly (no scalar-row chains, no separate broadcast matmuls), and
1/sqrt(var) is computed on DVE via Newton iteration so the ACT engine never
loads the sqrt table (a table-set switch costs ~2.7us).  Softmax
denominators are likewise accumulated as broadcast ones-matmuls; exp is
evaluated on paired PSUM banks ([128, 2*SQ] per instruction) to halve ACT
dispatch count in the attention phase.  The depthwise-conv residual and
h=x+attn adds run on GPSIMD (Pool).  The attention residual accumulator
(conv + Wo output, without x) is kept in bf16; x is added back in fp32 at
LN time.  All small constants ship in one packed DMA (dma_start dispatch
costs ~0.65us each on the sync engine).
"""

import math

import numpy as np
import ml_dtypes

import concourse.bass as bass
import concourse.tile as tile
from concourse import mybir
from concourse.alu_op_type import AluOpType
from bass_rust import ScopedClock

F32 = mybir.dt.float32
I32 = mybir.dt.int32
BF16 = mybir.dt.bfloat16
F16 = mybir.dt.float16
AF = mybir.ActivationFunctionType
OP = AluOpType

EPS = 1e-5
N_CORES = 8
MAGIC = 0x5F3759DF


class CFG:
    def __init__(self, S=1024, D=1024, F=4096, H=16):
        self.S, self.D, self.F, self.H = S, D, F, H
        self.DK = D // H              # head dim (must be 64)
        self.KD = D // 128            # feature tiles of model dim
        self.KF = F // 128            # feature tiles of ffn dim
        self.SQ = min(512, S)         # moving-dim chunk
        self.NQ = S // self.SQ
        self.NKT = S // 128           # key/sequence tiles
        self.VC = min(512, D)         # v-projection output chunk
        self.NVC = D // self.VC
        assert self.DK == 64 and H % 2 == 0


FULL = CFG()


def _split_excess_waits(nc, max_waits=1):
    """Walrus in this container rejects >2 sync waits per instruction.
    Hoist excess waits onto same-engine nops inserted just before."""
    cnt = 0
    for fn in nc.m.functions:
        for bb in fn.blocks:
            insts = list(bb.instructions)
            out = []
            for inst in insts:
                si = inst.sync_info
                waits = list(si.on_wait) if si and si.on_wait else []
                if len(waits) > max_waits:
                    extra = waits[:-max_waits]
                    si.on_wait = waits[-max_waits:]
                    for i in range(0, len(extra), max_waits):
                        cnt += 1
                        out.append(mybir.InstNoOp(
                            name=f"waitsplit{cnt}_{inst.name}",
                            engine=inst.engine, ins=[], outs=[],
                            sync_info=mybir.SyncInfo(
                                on_wait=extra[i:i + max_waits], on_update=[]),
                        ))
                out.append(inst)
            if cnt:
                bb.instructions = out
    return cnt


class _TC(tile.TileContext):
    """TileContext whose exit drain spreads semaphore waits over several
    sync-engine nops -- this container's walrus rejects >2 sync waits on a
    single CTRL instruction."""

    def __exit__(self, *a):
        r = super().__exit__(*a)
        _split_excess_waits(self.nc)
        return r

    def _drain_and_barrier(self, tick_clock, wait_clock):
        nc = self.nc
        drain_inst = nc.sync.drain()
        wait_clock.add_sem_waits(
            drain_inst.ins, ScopedClock({None: tick_clock.global_clock})
        )
        si = drain_inst.ins.sync_info
        waits = list(si.on_wait) if si and si.on_wait else []
        if len(waits) > 1:
            si.on_wait = waits[:1]
            for w in waits[1:]:
                nop = nc.sync.nop(nofuse=True)
                nsi = nop.ins.sync_info
                if nsi is None:
                    nop.ins.sync_info = mybir.SyncInfo(on_wait=[w], on_update=[])
                else:
                    nsi.on_wait = [w]
        nc.all_engine_barrier()
        popped = nc._tile_sem_poison_stack.pop()
        assert popped is self._sem_poison
        nc.clear_and_free_semaphores(list(self.sems.allocated().values()))
        nc.all_engine_barrier()


def emit(tc, cfg, io):
    nc = tc.nc
    S, D, F, H = cfg.S, cfg.D, cfg.F, cfg.H
    DK, KD, KF = cfg.DK, cfg.KD, cfg.KF
    SQ, NQ, NKT = cfg.SQ, cfg.NQ, cfg.NKT
    VC, NVC = cfg.VC, cfg.NVC
    HP = H // 2

    def qs(qc):
        return slice(qc * SQ, (qc + 1) * SQ)

    pool = tc.alloc_tile_pool

    # ======== pools: alloc order is EXACT reverse death order per side ====
    consts = pool(name="consts", bufs=1)
    tmpp = pool(name="tmp", bufs=1)
    vecp = pool(name="vec", bufs=1)
    sqp = pool(name="sq", bufs=1)
    smallp = pool(name="small", bufs=1)
    stgp = pool(name="stg", bufs=1)
    wd = pool(name="wd", bufs=1)
    rp = pool(name="racc", bufs=1)           # bf16 conv+Wo residual
    xpp = pool(name="xp", bufs=1)
    xbfp = pool(name="xbf", bufs=1)          # x bf16, then reused as h
    vtp = pool(name="vt", bufs=1, side="right")
    wvp = pool(name="wv", bufs=1, side="right")
    xhp = pool(name="xh", bufs=1, side="right")  # fp16 x staging
    aop = pool(name="aop", bufs=1)
    qkp = pool(name="qk", bufs=1)
    expp = pool(name="expT", bufs=1)
    psum = pool(name="ps", bufs=1, space="PSUM")

    # ---------------- constants (one packed DMA) ----------------
    cpk = consts.tile([128, 16 * KD + 2 * KF], F32, name="cpk", tag="cpk")
    ct = {}
    off = 0
    for cname in ("bq", "bk", "bocb", "cw0", "cw1", "cw2", "b3t", "bgt",
                  "lnag", "lnab", "n1g", "n1b", "n2g", "n2b", "bdla", "bdn1"):
        ct[cname] = cpk[:, off:off + KD]
        off += KD
    for cname in ("b1t", "b2t"):
        ct[cname] = cpk[:, off:off + KF]
        off += KF
    ones_bf = consts.tile([128, 128], BF16, name="onesbf", tag="onesbf")
    nc.vector.memset(ones_bf[:], 1.0)
    bvr_bf = consts.tile([1, D], BF16, name="bvrbf", tag="bvrbf")
    nc.sync.dma_start(bvr_bf[:], io["bvr"][:, :])
    nc.sync.dma_start(cpk[:], io["cpk"][:, :])

    def ps_tile():
        return psum.tile([128, SQ], F32, name="ps", tag="ps", bufs=2)

    def pp_tile():
        return psum.tile([128, 2, SQ], F32, name="pp", tag="pp", bufs=3)

    # ---------------- x + wv loads, staged for earliest v-proj ------------
    xp, xbf = [], []
    for kt in range(KD):
        t = xpp.tile([128, S + 2], F32, name=f"xp{kt}", tag=f"xp{kt}")
        xp.append(t)
        b = xbfp.tile([128, S], BF16, name=f"xbf{kt}", tag=f"xbf{kt}")
        xbf.append(b)
    wv = [wvp.tile([128, D], BF16, name=f"wv{kt}", tag=f"wv{kt}")
          for kt in range(KD)]
    for kt in range(KD):
        nc.sync.dma_start(wv[kt][:], io["wv"][kt * 128:(kt + 1) * 128, :])
    # x arrives fp16; stage through a rotating SBUF tile, then fan out to
    # the bf16 matmul copy (DVE) and the fp32 conv/LN copy (DVE).
    def stage_x(kt, c0, c1):
        xh = xhp.tile([128, 512], F16, name="xh", tag="xh",
                      bufs=3)[:, :c1 - c0]
        nc.sync.dma_start(xh, io["xT"][kt * 128:(kt + 1) * 128, c0:c1])
        nc.vector.tensor_copy(xbf[kt][:, c0:c1], xh)
        nc.vector.tensor_copy(xp[kt][:, 1 + c0:1 + c1], xh)

    # first seq tile of x (enables v-proj st=0), then the rest
    for kt in range(KD):
        stage_x(kt, 0, 128)
    for kt in range(KD):
        stage_x(kt, 128, SQ)
    for kt in range(KD):
        stage_x(kt, SQ, S)
        nc.vector.memset(xp[kt][:, 0:1], 0.0)
        nc.vector.memset(xp[kt][:, S + 1:S + 2], 0.0)

    # ---------------- v projection (x stationary, wv moving) --------------
    v_t = [vtp.tile([128, D], BF16, name=f"vt{st}", tag=f"vt{st}")
           for st in range(NKT)]
    for st in range(NKT):
        for ec in range(NVC):
            ps = ps_tile()
            for kt in range(KD):
                nc.tensor.matmul(ps[:, 0:VC],
                                 xbf[kt][:, st * 128:(st + 1) * 128],
                                 wv[kt][:, ec * VC:(ec + 1) * VC],
                                 start=(kt == 0), stop=False)
            # bv bias via K=1 ones-row matmul into the same accumulation
            nc.tensor.matmul(ps[:, 0:VC], ones_bf[0:1, 0:128],
                             bvr_bf[0:1, ec * VC:(ec + 1) * VC],
                             start=False, stop=True)
            nc.vector.tensor_copy(v_t[st][:, ec * VC:(ec + 1) * VC],
                                  ps[:, 0:VC])
    xhp.release()
    wvp.release()

    # ---------------- conv residual on GPSIMD: racc = 0.3*conv ------------
    # (x itself is NOT accumulated here; it is added back in fp32 at LN
    #  time.  racc is bf16: it only ever holds conv + attention output.)
    racc = [rp.tile([128, S], BF16, name=f"ra{kt}", tag=f"ra{kt}")
            for kt in range(KD)]
    for kt in range(KD):
        for qc in range(NQ):
            o = qc * SQ
            xl, xc, xr = (xp[kt][:, o:o + SQ], xp[kt][:, o + 1:o + SQ + 1],
                          xp[kt][:, o + 2:o + SQ + 2])
            t1 = tmpp.tile([128, SQ], F32, name="lnt", tag="lnt", bufs=4)
            nc.vector.tensor_scalar(t1[:], xl, ct["cw0"][:, kt:kt + 1], None,
                                    op0=OP.mult)
            t2 = tmpp.tile([128, SQ], F32, name="lnt", tag="lnt", bufs=4)
            nc.vector.scalar_tensor_tensor(t2[:], xc, ct["cw1"][:, kt:kt + 1],
                                           t1[:], op0=OP.mult, op1=OP.add)
            nc.vector.scalar_tensor_tensor(racc[kt][:, qs(qc)], xr,
                                           ct["cw2"][:, kt:kt + 1], t2[:],
                                           op0=OP.mult, op1=OP.add)

    # ---------------- q/k projection (paired psum: both qc at once) -------
    q_fm = [qkp.tile([128, S], BF16, name=f"q{m}", tag=f"q{m}")
            for m in range(KD)]
    k_fm = [qkp.tile([128, S], BF16, name=f"k{m}", tag=f"k{m}")
            for m in range(KD)]

    def wd_tile():
        return wd.tile([128, D], BF16, name="wd", tag="wd", bufs=3)

    def qk_proj(m):
        for wname, bias, dst in (("wqs", ct["bq"], q_fm),
                                 ("wks", ct["bk"], k_fm)):
            wt = wd_tile()
            nc.sync.dma_start(wt[:], io[wname][:, m * D:(m + 1) * D])
            pp = pp_tile()
            for qc in range(NQ):
                for kt in range(KD):
                    nc.tensor.matmul(pp[:, qc, :],
                                     wt[:, kt * 128:(kt + 1) * 128],
                                     xbf[kt][:, qs(qc)],
                                     start=(kt == 0), stop=(kt == KD - 1))
            nc.scalar.activation(dst[m][:], pp[:], AF.Identity,
                                 bias=bias[:, m:m + 1])

    # ---------------- attention ----------------
    inv_sqrt_dk = 1.0 / math.sqrt(DK)
    _ao = []

    def attn(hp, qc):
        attnout = _ao
        h0, h1 = 2 * hp, 2 * hp + 1
        eps_ = []
        for kt in range(NKT):
            pp = pp_tile()
            nc.tensor.matmul(pp[:, 0, :],
                             k_fm[hp][0:64, kt * 128:(kt + 1) * 128],
                             q_fm[hp][0:64, qs(qc)], start=True, stop=True)
            nc.tensor.matmul(pp[:, 1, :],
                             k_fm[hp][64:128, kt * 128:(kt + 1) * 128],
                             q_fm[hp][64:128, qs(qc)], start=True, stop=True)
            ep = expp.tile([128, 2, SQ], BF16, name="exp", tag="exp", bufs=9)
            nc.scalar.activation(ep[:], pp[:], AF.Exp, scale=inv_sqrt_dk)
            eps_.append(ep)
        den_t, U_t = ps_tile(), ps_tile()
        den, U = den_t[:, :], U_t[:, :]
        for kt in range(NKT):
            nc.tensor.matmul(den[0:64, :], ones_bf[:, 0:64], eps_[kt][:, 0, :],
                             start=(kt == 0), stop=(kt == NKT - 1))
        for kt in range(NKT):
            nc.tensor.matmul(den[64:128, :], ones_bf[:, 0:64],
                             eps_[kt][:, 1, :],
                             start=(kt == 0), stop=(kt == NKT - 1))
        for kt in range(NKT):
            nc.tensor.matmul(U[0:64, :], v_t[kt][:, h0 * DK:(h0 + 1) * DK],
                             eps_[kt][:, 0, :],
                             start=(kt == 0), stop=(kt == NKT - 1))
        for kt in range(NKT):
            nc.tensor.matmul(U[64:128, :], v_t[kt][:, h1 * DK:(h1 + 1) * DK],
                             eps_[kt][:, 1, :],
                             start=(kt == 0), stop=(kt == NKT - 1))
        recb = smallp.tile([128, SQ], F32, name="recb", tag="recb", bufs=2)
        nc.vector.reciprocal(recb[:], den)
        nc.vector.tensor_tensor(attnout[hp][:, qs(qc)], U, recb[:],
                                op=OP.mult)

    # ---------------- Wo accumulate into racc ----------------
    def wo_step(m, qc):
        attnout = _ao
        wt = wd_tile()
        nc.sync.dma_start(wt[:], io["wos"][:, m * D:(m + 1) * D])
        ps = ps_tile()
        for kt in range(KD):
            nc.tensor.matmul(ps[:], wt[:, kt * 128:(kt + 1) * 128],
                             attnout[kt][:, qs(qc)],
                             start=(kt == 0), stop=(kt == KD - 1))
        nc.vector.scalar_tensor_tensor(racc[m][:, qs(qc)], ps[:],
                                       ct["bocb"][:, m:m + 1],
                                       racc[m][:, qs(qc)],
                                       op0=OP.add, op1=OP.add)

    # ---------------- layernorm helpers (broadcast stats) ----------------
    def vtile(tag, dt=F32):
        return vecp.tile([128, SQ], dt, name=tag, tag=tag, bufs=1)

    def ln_stats(cast_fn):
        """Returns psum pair (ssum at [:,0,:], ssq at [:,1,:])."""
        sp = pp_tile()
        for kt in range(KD):
            rbf = cast_fn(kt)
            sq_t = sqp.tile([128, SQ], BF16, name="sq", tag="sq", bufs=2)
            nc.vector.tensor_tensor(sq_t[:], rbf[:], rbf[:], op=OP.mult)
            st_, sp_ = (kt == 0), (kt == KD - 1)
            nc.tensor.matmul(sp[:, 0, :], ones_bf[:, 0:128], rbf[:],
                             start=st_, stop=sp_)
            nc.tensor.matmul(sp[:, 1, :], ones_bf[:, 0:128], sq_t[:],
                             start=st_, stop=sp_)
        return sp

    def ln_finish(sp, c0=0, c1=None):
        """mu, rstd [128, c1-c0] from the stats pair (Newton rsqrt on DVE)."""
        c1 = SQ if c1 is None else c1
        mu_t, var_t, rst_t = vtile("vmu"), vtile("vvar"), vtile("vrst")
        mu, var, rst = mu_t[:, c0:c1], var_t[:, c0:c1], rst_t[:, c0:c1]
        nc.vector.tensor_scalar_mul(mu, sp[:, 0, c0:c1], 1.0 / D)
        nc.vector.tensor_tensor(var, mu, mu, op=OP.mult)
        nc.vector.scalar_tensor_tensor(var, sp[:, 1, c0:c1], 1.0 / D, var,
                                       op0=OP.mult, op1=OP.subtract)
        nc.vector.tensor_scalar_add(var, var, EPS)
        rst_i = rst_t[:].bitcast(I32)[:, c0:c1]
        var_i = var_t[:].bitcast(I32)[:, c0:c1]
        nc.vector.tensor_scalar(rst_i, var_i, 1, None,
                                op0=OP.arith_shift_right)
        nc.vector.tensor_scalar(rst_i, rst_i, -1, MAGIC, op0=OP.mult,
                                op1=OP.add)
        for _ in range(2):
            t = vtile("vnt")[:, c0:c1]
            nc.vector.tensor_tensor(t, rst, rst, op=OP.mult)
            nc.vector.tensor_tensor(t, t, var, op=OP.mult)
            nc.vector.tensor_scalar(t, t, -0.5, 1.5, op0=OP.mult,
                                    op1=OP.add)
            nc.vector.tensor_tensor(rst, rst, t, op=OP.mult)
        return mu, rst

    def ln_norm_simple(src_fn, mu, rst, write_out, c0=0, c1=None):
        c1 = SQ if c1 is None else c1
        for kt in range(KD):
            t1 = tmpp.tile([128, SQ], F32, name="lnt", tag="lnt", bufs=4)
            nc.vector.tensor_tensor(t1[:, c0:c1], src_fn(kt), mu,
                                    op=OP.subtract)
            nc.vector.tensor_tensor(t1[:, c0:c1], t1[:, c0:c1], rst,
                                    op=OP.mult)
            write_out(kt, t1)

    # ---------------- LN chains ----------------
    def lna_r2(qc):
        """lna on (x + racc); then r2 = x + lna_out accumulated into xp."""
        def cast_lna(kt):
            rbf = sqp.tile([128, SQ], BF16, name="rbf", tag="rbf", bufs=2)
            nc.gpsimd.tensor_tensor(
                rbf[:], xp[kt][:, 1 + qc * SQ:1 + (qc + 1) * SQ],
                racc[kt][:, qs(qc)], op=OP.add)
            return rbf

        sp = ln_stats(cast_lna)
        mu, rst = ln_finish(sp)
        for kt in range(KD):
            xs = xp[kt][:, 1 + qc * SQ:1 + (qc + 1) * SQ]
            t1 = tmpp.tile([128, SQ], F32, name="lnt", tag="lnt", bufs=4)
            nc.vector.tensor_tensor(t1[:], xs, racc[kt][:, qs(qc)], op=OP.add)
            nc.vector.tensor_tensor(t1[:], t1[:], mu[:], op=OP.subtract)
            nc.vector.tensor_tensor(t1[:], t1[:], rst[:], op=OP.mult)
            t2 = tmpp.tile([128, SQ], F32, name="lnt", tag="lnt", bufs=4)
            nc.scalar.activation(t2[:], t1[:], AF.Identity,
                                 bias=ct["lnab"][:, kt:kt + 1],
                                 scale=ct["lnag"][:, kt:kt + 1])
            nc.gpsimd.tensor_tensor(xs, xs, t2[:], op=OP.add)

    h_holder = []

    def n1_chain(qc):
        h_bf = h_holder

        def cast_n1(kt):
            rbf = sqp.tile([128, SQ], BF16, name="rbf", tag="rbf", bufs=2)
            nc.gpsimd.tensor_copy(rbf[:],
                                  xp[kt][:, 1 + qc * SQ:1 + (qc + 1) * SQ])
            return rbf

        sp = ln_stats(cast_n1)
        mu, rst = ln_finish(sp)

        def w_n1(kt, t1):
            nc.scalar.activation(h_bf[kt][:, qs(qc)], t1[:], AF.Identity,
                                 bias=ct["n1b"][:, kt:kt + 1],
                                 scale=ct["n1g"][:, kt:kt + 1])

        ln_norm_simple(
            lambda kt: xp[kt][:, 1 + qc * SQ:1 + (qc + 1) * SQ], mu, rst, w_n1)

    # ========================================================================
    # schedule
    # ========================================================================
    for m in range(KD):
        qk_proj(m)
    _ao.extend(aop.tile([128, S], BF16, name=f"ao{m}", tag=f"ao{m}")
               for m in range(KD))
    h_holder.extend(xbf)    # reuse the dead x-bf16 tiles as h storage
    for hp in range(HP):
        attn(hp, 0)
    for m in range(KD):
        wo_step(m, 0)
    lna_r2(0)               # chains run on DVE/Pool under attn(qc1) below
    for hp in range(HP // 2):
        attn(hp, 1)
    n1_chain(0)
    for hp in range(HP // 2, HP):
        attn(hp, 1)
    expp.release()
    qkp.release()
    vtp.release()

    for m in range(KD):
        wo_step(m, 1)
    aop.release()
    lna_r2(1)               # chain overlaps FFN-W1(qc0)

    f1p = pool(name="ffn1", bufs=1, side="right")

    def ffn_w1(qc, mid=None):
        f1t = []
        for m in range(KF):
            if m == 8 and mid is not None:
                mid()
            wt = wd_tile()
            nc.sync.dma_start(wt[:], io["w1s"][:, m * D:(m + 1) * D])
            ps = ps_tile()
            for kt in range(KD):
                nc.tensor.matmul(ps[:], wt[:, kt * 128:(kt + 1) * 128],
                                 h_holder[kt][:, qs(qc)],
                                 start=(kt == 0), stop=(kt == KD - 1))
            t = f1p.tile([128, SQ], BF16, name=f"f1_{m}", tag=f"f1_{m}")
            nc.scalar.activation(t[:], ps[:], AF.Gelu,
                                 bias=ct["b1t"][:, m:m + 1])
            f1t.append(t)
        return f1t

    f1_qc0 = ffn_w1(0, mid=lambda: n1_chain(1))

    wf = pool(name="wf", bufs=1)
    fop = pool(name="fout", bufs=1)
    f2p = pool(name="ffn2", bufs=1, side="right")
    fout = [fop.tile([128, SQ], F32, name=f"fo{m}", tag=f"fo{m}")
            for m in range(KD)]

    def wf_tile():
        """Half of a W2/W3 contraction block: [128, F//2]."""
        return wf.tile([128, F // 2], BF16, name="wf", tag="wf", bufs=3)

    def big_mm(ps, wname, m, rhs_tiles):
        """Accumulate over KF k-tiles streaming weights in two half tiles."""
        for h in range(2):
            wt = wf_tile()
            nc.sync.dma_start(wt[:], io[wname][:, m * F + h * (F // 2):
                                               m * F + (h + 1) * (F // 2)])
            for j in range(KF // 2):
                kt = h * (KF // 2) + j
                nc.tensor.matmul(ps, wt[:, j * 128:(j + 1) * 128],
                                 rhs_tiles[kt][:],
                                 start=(kt == 0), stop=(kt == KF - 1))

    def ffn_rest(qc, f1t, mid=None):
        h_bf = h_holder
        f2t = []
        for m in range(KF):
            if m == 8 and mid is not None:
                mid()
            ps = ps_tile()
            big_mm(ps[:], "w2s", m, f1t)
            t = f2p.tile([128, SQ], BF16, name=f"f2_{m}", tag=f"f2_{m}")
            nc.scalar.activation(t[:], ps[:], AF.Gelu,
                                 bias=ct["b2t"][:, m:m + 1])
            f2t.append(t)
        # W3 + gate, with incremental n2 statistics per output tile
        n2sp = pp_tile()
        for m in range(KD):
            wtg = wd_tile()
            nc.sync.dma_start(wtg[:], io["wgs"][:, m * D:(m + 1) * D])
            gp = pp_tile()
            psg, ps3 = gp[:, 0, :], gp[:, 1, :]
            for kt in range(KD):
                nc.tensor.matmul(psg, wtg[:, kt * 128:(kt + 1) * 128],
                                 h_bf[kt][:, qs(qc)],
                                 start=(kt == 0), stop=(kt == KD - 1))
            gat = tmpp.tile([128, SQ], BF16, name="gat", tag="gat", bufs=2)
            nc.scalar.activation(gat[:], psg, AF.Sigmoid,
                                 bias=ct["bgt"][:, m:m + 1])
            big_mm(ps3, "w3s", m, f2t)
            t = tmpp.tile([128, SQ], F32, name="f3t", tag="f3t", bufs=2)
            nc.vector.scalar_tensor_tensor(t[:], ps3, ct["b3t"][:, m:m + 1],
                                           gat[:], op0=OP.add, op1=OP.mult)
            nc.vector.tensor_tensor(fout[m][:], t[:],
                                    h_bf[m][:, qs(qc)], op=OP.add)
            # incremental n2 stats for this feature tile
            rbf = sqp.tile([128, SQ], BF16, name="rbf", tag="rbf", bufs=2)
            nc.gpsimd.tensor_copy(rbf[:], fout[m][:])
            sq_t = sqp.tile([128, SQ], BF16, name="sq", tag="sq", bufs=2)
            nc.scalar.square(sq_t[:], rbf[:])
            st_, sp_ = (m == 0), (m == KD - 1)
            nc.tensor.matmul(n2sp[:, 0, :], ones_bf[:, 0:128], rbf[:],
                             start=st_, stop=sp_)
            nc.tensor.matmul(n2sp[:, 1, :], ones_bf[:, 0:128], sq_t[:],
                             start=st_, stop=sp_)
        return n2sp

    def emit_n2(qc, n2sp):
        mu, rst = ln_finish(n2sp)

        def w_n2(kt, t1):
            stg = stgp.tile([128, SQ], F16, name="stg", tag="stg", bufs=2)
            nc.scalar.activation(stg[:], t1[:], AF.Identity,
                                 bias=ct["n2b"][:, kt:kt + 1],
                                 scale=ct["n2g"][:, kt:kt + 1])
            nc.sync.dma_start(io["outT"][kt * 128:(kt + 1) * 128, qs(qc)],
                              stg[:])

        ln_norm_simple(lambda kt: fout[kt][:], mu, rst, w_n2)

    n2sp0 = ffn_rest(0, f1_qc0)
    emit_n2(0, n2sp0)
    f1_qc1 = ffn_w1(1)
    n2sp1 = ffn_rest(1, f1_qc1)
    emit_n2(1, n2sp1)

    f2p.release()
    f1p.release()
    fop.release()
    wf.release()
    xbfp.release()
    xpp.release()
    rp.release()
    wd.release()
    stgp.release()
    smallp.release()
    sqp.release()
    vecp.release()
    tmpp.release()
    consts.release()
    psum.release()


# ------------------------------------------------------------------
# host side
# ------------------------------------------------------------------

def _shuffle_w(w):
    """[K, E] -> [128, (E//128)*K] bf16 so that slice [:, m*K:(m+1)*K]
    viewed as [128, K//128, 128] gives lhsT tiles w[kt*128+p, m*128+c]."""
    K, E = w.shape
    r = np.asarray(w).reshape(K // 128, 128, E // 128, 128).transpose(1, 2, 0, 3)
    return np.ascontiguousarray(r.reshape(128, (E // 128) * K)).astype(
        ml_dtypes.bfloat16)


def _ptable(b):
    """[E] -> [128, E//128] per-partition scalar table."""
    return np.ascontiguousarray(np.asarray(b, np.float32).reshape(-1, 128).T)


def _declare_io(nc, cfg, consts=None):
    """xT/outT are per-call I/O; every weight tensor is embedded in the
    NEFF as a Const DRAM tensor (DMA'd to HBM once at model-load time) so
    repeated executions only ship the activation tensors."""
    S, D, F, KD, KF = cfg.S, cfg.D, cfg.F, cfg.KD, cfg.KF
    io = {}

    def inp(name, shape, dt):
        if consts is not None and name in consts:
            arr = np.ascontiguousarray(consts[name])
            assert list(arr.shape) == list(shape), (name, arr.shape, shape)
            io[name] = nc.inline_tensor(arr, name=name).ap()
        else:
            io[name] = nc.dram_tensor(name, shape, dt, kind="ExternalInput").ap()

    inp("xT", [D, S], F16)
    inp("wqs", [128, KD * D], BF16)
    inp("wks", [128, KD * D], BF16)
    inp("wv", [D, D], BF16)
    inp("wos", [128, KD * D], BF16)
    inp("w1s", [128, KF * D], BF16)
    inp("w2s", [128, KF * F], BF16)
    inp("wgs", [128, KD * D], BF16)
    inp("w3s", [128, KD * F], BF16)
    inp("cpk", [128, 16 * KD + 2 * KF], F32)
    inp("bvr", [1, D], BF16)
    io["outT"] = nc.dram_tensor("outT", [D, S], F16, kind="ExternalOutput").ap()
    return io


def build_shared_inputs(inputs, cfg):
    """Everything except xT (identical across cores)."""
    f32 = np.float32
    g = {k: np.asarray(v) for k, v in inputs.items()}
    ptabs = [
        _ptable(g["bq"]), _ptable(g["bk"]),
        _ptable(np.asarray(g["bo"], f32) + 0.3 * np.asarray(g["conv_b"], f32)),
        _ptable(0.3 * np.asarray(g["conv_w"], f32)[:, 0]),
        _ptable(0.3 * np.asarray(g["conv_w"], f32)[:, 1]),
        _ptable(0.3 * np.asarray(g["conv_w"], f32)[:, 2]),
        _ptable(g["b3"]), _ptable(g["bg"]),
        _ptable(g["lna_g"]), _ptable(g["lna_b"]),
        _ptable(g["n1_g"]), _ptable(g["n1_b"]),
        _ptable(g["n2_g"]), _ptable(g["n2_b"]),
        _ptable(np.asarray(g["lna_b"], f32)
                / np.where(np.abs(np.asarray(g["lna_g"], f32)) < 1e-20,
                           1.0, np.asarray(g["lna_g"], f32))),
        _ptable(np.asarray(g["n1_b"], f32)
                / np.where(np.abs(np.asarray(g["n1_g"], f32)) < 1e-20,
                           1.0, np.asarray(g["n1_g"], f32))),
        _ptable(g["b1"]), _ptable(g["b2"]),
    ]
    sh = {
        "wqs": _shuffle_w(g["Wq"]), "wks": _shuffle_w(g["Wk"]),
        "wv": np.ascontiguousarray(g["Wv"]).astype(ml_dtypes.bfloat16),
        "wos": _shuffle_w(g["Wo"]), "w1s": _shuffle_w(g["W1"]),
        "w2s": _shuffle_w(g["W2"]), "w3s": _shuffle_w(g["W3"]),
        "wgs": _shuffle_w(g["Wg"]),
        "cpk": np.ascontiguousarray(np.concatenate(ptabs, axis=1)),
        "bvr": np.ascontiguousarray(
            np.asarray(g["bv"], f32).reshape(1, cfg.D)).astype(
                ml_dtypes.bfloat16),
    }
    return sh


_CACHE = {}


def _weights_fingerprint(inputs):
    """Content hash of every non-x input. The NEFF embeds the weights, so
    a changed weight set must rebuild (and recompile) the kernel."""
    import hashlib
    h = hashlib.blake2b(digest_size=16)
    for k in sorted(inputs):
        if k == "x":
            continue
        a = np.ascontiguousarray(inputs[k])
        h.update(k.encode())
        h.update(str(a.shape).encode())
        h.update(str(a.dtype).encode())
        h.update(a.tobytes())
    return h.hexdigest()


def _get_nc(inputs):
    key = _weights_fingerprint(inputs)
    ent = _CACHE.get("nc")
    if ent is None or ent[0] != key:
        shared = build_shared_inputs(inputs, FULL)
        nc = bass.Bass("TRN2", target_bir_lowering=False, debug=False)
        io = _declare_io(nc, FULL, consts=shared)
        with _TC(nc) as tc:
            emit(tc, FULL, io)
        _CACHE["nc"] = (key, nc)
    return _CACHE["nc"][1]


def _get_exec(inputs):
    """Persistent jitted executor: one bass_exec custom call shard_mapped
    over the 8 cores.  Rebuilt only when the weight set changes."""
    import jax
    from jax.sharding import Mesh, PartitionSpec
    from jax.experimental.shard_map import shard_map
    from concourse import bass2jax

    key = _weights_fingerprint(inputs)
    ent = _CACHE.get("exec")
    if ent is not None and ent[0] == key:
        return ent[1]

    bass2jax.install_neuronx_cc_hook()
    nc = _get_nc(inputs)
    pid = nc.partition_id_tensor.name if nc.partition_id_tensor else None
    in_names, out_names, out_avals = [], [], []
    for alloc in nc.m.functions[0].allocations:
        if not isinstance(alloc, mybir.MemoryLocationSet):
            continue
        name = alloc.memorylocations[0].name
        if alloc.kind == "ExternalInput":
            if name != pid:
                in_names.append(name)
        elif alloc.kind == "ExternalOutput":
            out_names.append(name)
            out_avals.append(jax.core.ShapedArray(
                tuple(alloc.tensor_shape), mybir.dt.np(alloc.dtype)))
    all_names = in_names + ([pid] if pid is not None else [])
    assert in_names == ["xT"] and out_names == ["outT"]

    def _body(*args):
        operands = list(args)
        if pid is not None:
            operands.append(bass2jax.partition_id_tensor())
        return tuple(bass2jax._bass_exec_p.bind(
            *operands, out_avals=tuple(out_avals), in_names=tuple(all_names),
            out_names=tuple(out_names), lowering_input_output_aliases=(),
            sim_require_finite=True, sim_require_nnan=True, nc=nc))

    mesh = Mesh(np.asarray(jax.devices()[:N_CORES]), ("core",))
    sharded = jax.jit(
        shard_map(_body, mesh=mesh, in_specs=(PartitionSpec("core"),),
                  out_specs=(PartitionSpec("core"),), check_rep=False),
        keep_unused=True)
    _CACHE["exec"] = (key, sharded)
    return sharded


def kernel(**inputs):
    import jax

    cfg = FULL
    sharded = _get_exec(inputs)
    x = np.asarray(inputs["x"], dtype=np.float32)
    B = x.shape[0]
    assert B == N_CORES
    xc = np.concatenate([x[b].T.astype(np.float16, order="C")
                         for b in range(B)], axis=0)
    out = sharded(jax.device_put(xc))
    o0 = np.asarray(out[0]).astype(np.float32).reshape(B, cfg.D, cfg.S)
    return np.ascontiguousarray(o0.transpose(0, 2, 1))



# revision 17
# speedup vs baseline: 1.0230x; 1.0036x over previous
"""Trainium2 Bass kernel for an enhanced transformer layer.

Strategy: data-parallel over batch (B=8 -> one batch element per NeuronCore,
no collectives).  Activations are kept feature-major ([D, S] with the
contraction dim on partitions) so every linear layer consumes weights in
natural [K, E] layout as the stationary operand.  Matmuls run in bf16 with
fp32 PSUM accumulation.

I/O strategy: all weight tensors are embedded in the NEFF as Const DRAM
tensors (DMA'd to HBM once at model-load time), so per-execution bindings
are just xT (fp16 in) and outT (fp16 out) -- 2 MB each per core.  The NEFF
is rebuilt (keyed on a content hash of the weights) if kernel() is called
with a different weight set.  fp16 transport adds ~2^-11 relative
quantization noise on x and out, negligible against the bf16 matmul noise.

Schedule: the layer is software-pipelined over the two S/2 column chunks
(qc) so LayerNorm / softmax vector work overlaps matmul streams:
  v-proj (staged x arrival) -> qk-proj -> attn(qc0) -> [attn(hp, qc1) |
  Wo(qc0)] -> lna(qc0) | Wo(qc1).a -> n1(qc0) | Wo(qc1).b ->
  FFN-W1(qc0) | lna(qc1) -> FFN-W2(qc0) | n1(qc1) -> FFN-W3/gate(qc0) with
  incremental n2(qc0) stats -> n2(qc0) | FFN(qc1) -> n2(qc1).
LayerNorm statistics use M=128 ones-matmuls that produce row-broadcast sums
directly (no scalar-row chains, no separate broadcast matmuls), and
1/sqrt(var) is computed on DVE via Newton iteration so the ACT engine never
loads the sqrt table (a table-set switch costs ~2.7us).  Softmax
denominators are likewise accumulated as broadcast ones-matmuls; exp is
evaluated on paired PSUM banks ([128, 2*SQ] per instruction) to halve ACT
dispatch count in the attention phase.  The depthwise-conv residual and
h=x+attn adds run on GPSIMD (Pool).  The attention residual accumulator
(conv + Wo output, without x) is kept in bf16; x is added back in fp32 at
LN time.  All small constants ship in one packed DMA (dma_start dispatch
costs ~0.65us each on the sync engine).
"""

import math

import numpy as np
import ml_dtypes

import concourse.bass as bass
import concourse.tile as tile
from concourse import mybir
from concourse.alu_op_type import AluOpType
from bass_rust import ScopedClock

F32 = mybir.dt.float32
I32 = mybir.dt.int32
BF16 = mybir.dt.bfloat16
F16 = mybir.dt.float16
AF = mybir.ActivationFunctionType
OP = AluOpType

EPS = 1e-5
N_CORES = 8
MAGIC = 0x5F3759DF


class CFG:
    def __init__(self, S=1024, D=1024, F=4096, H=16):
        self.S, self.D, self.F, self.H = S, D, F, H
        self.DK = D // H              # head dim (must be 64)
        self.KD = D // 128            # feature tiles of model dim
        self.KF = F // 128            # feature tiles of ffn dim
        self.SQ = min(512, S)         # moving-dim chunk
        self.NQ = S // self.SQ
        self.NKT = S // 128           # key/sequence tiles
        self.VC = min(512, D)         # v-projection output chunk
        self.NVC = D // self.VC
        assert self.DK == 64 and H % 2 == 0


FULL = CFG()


def _split_excess_waits(nc, max_waits=1):
    """Walrus in this container rejects >2 sync waits per instruction.
    Hoist excess waits onto same-engine nops inserted just before."""
    cnt = 0
    for fn in nc.m.functions:
        for bb in fn.blocks:
            insts = list(bb.instructions)
            out = []
            for inst in insts:
                si = inst.sync_info
                waits = list(si.on_wait) if si and si.on_wait else []
                if len(waits) > max_waits:
                    extra = waits[:-max_waits]
                    si.on_wait = waits[-max_waits:]
                    for i in range(0, len(extra), max_waits):
                        cnt += 1
                        out.append(mybir.InstNoOp(
                            name=f"waitsplit{cnt}_{inst.name}",
                            engine=inst.engine, ins=[], outs=[],
                            sync_info=mybir.SyncInfo(
                                on_wait=extra[i:i + max_waits], on_update=[]),
                        ))
                out.append(inst)
            if cnt:
                bb.instructions = out
    return cnt


class _TC(tile.TileContext):
    """TileContext whose exit drain spreads semaphore waits over several
    sync-engine nops -- this container's walrus rejects >2 sync waits on a
    single CTRL instruction."""

    def __exit__(self, *a):
        r = super().__exit__(*a)
        _split_excess_waits(self.nc)
        return r

    def _drain_and_barrier(self, tick_clock, wait_clock):
        nc = self.nc
        drain_inst = nc.sync.drain()
        wait_clock.add_sem_waits(
            drain_inst.ins, ScopedClock({None: tick_clock.global_clock})
        )
        si = drain_inst.ins.sync_info
        waits = list(si.on_wait) if si and si.on_wait else []
        if len(waits) > 1:
            si.on_wait = waits[:1]
            for w in waits[1:]:
                nop = nc.sync.nop(nofuse=True)
                nsi = nop.ins.sync_info
                if nsi is None:
                    nop.ins.sync_info = mybir.SyncInfo(on_wait=[w], on_update=[])
                else:
                    nsi.on_wait = [w]
        nc.all_engine_barrier()
        popped = nc._tile_sem_poison_stack.pop()
        assert popped is self._sem_poison
        nc.clear_and_free_semaphores(list(self.sems.allocated().values()))
        nc.all_engine_barrier()


def emit(tc, cfg, io):
    nc = tc.nc
    S, D, F, H = cfg.S, cfg.D, cfg.F, cfg.H
    DK, KD, KF = cfg.DK, cfg.KD, cfg.KF
    SQ, NQ, NKT = cfg.SQ, cfg.NQ, cfg.NKT
    VC, NVC = cfg.VC, cfg.NVC
    HP = H // 2

    def qs(qc):
        return slice(qc * SQ, (qc + 1) * SQ)

    pool = tc.alloc_tile_pool

    # ======== pools: alloc order is EXACT reverse death order per side ====
    consts = pool(name="consts", bufs=1)
    tmpp = pool(name="tmp", bufs=1)
    vecp = pool(name="vec", bufs=1)
    sqp = pool(name="sq", bufs=1)
    smallp = pool(name="small", bufs=1)
    stgp = pool(name="stg", bufs=1)
    wd = pool(name="wd", bufs=1)
    rp = pool(name="racc", bufs=1)           # bf16 conv+Wo residual
    xpp = pool(name="xp", bufs=1)
    xbfp = pool(name="xbf", bufs=1)          # x bf16, then reused as h
    vtp = pool(name="vt", bufs=1, side="right")
    wvp = pool(name="wv", bufs=1, side="right")
    xhp = pool(name="xh", bufs=1, side="right")  # fp16 x staging
    aop = pool(name="aop", bufs=1)
    qkp = pool(name="qk", bufs=1)
    expp = pool(name="expT", bufs=1)
    psum = pool(name="ps", bufs=1, space="PSUM")

    # ---------------- constants (one packed DMA) ----------------
    cpk = consts.tile([128, 16 * KD + 2 * KF], F32, name="cpk", tag="cpk")
    ct = {}
    off = 0
    for cname in ("bq", "bk", "bocb", "cw0", "cw1", "cw2", "b3t", "bgt",
                  "lnag", "lnab", "n1g", "n1b", "n2g", "n2b", "bdla", "bdn1"):
        ct[cname] = cpk[:, off:off + KD]
        off += KD
    for cname in ("b1t", "b2t"):
        ct[cname] = cpk[:, off:off + KF]
        off += KF
    ones_bf = consts.tile([128, 128], BF16, name="onesbf", tag="onesbf")
    nc.vector.memset(ones_bf[:], 1.0)
    bvr_bf = consts.tile([1, D], BF16, name="bvrbf", tag="bvrbf")
    nc.scalar.dma_start(bvr_bf[:], io["bvr"][:, :])
    nc.scalar.dma_start(cpk[:], io["cpk"][:, :])

    def ps_tile():
        return psum.tile([128, SQ], F32, name="ps", tag="ps", bufs=2)

    def pp_tile():
        return psum.tile([128, 2, SQ], F32, name="pp", tag="pp", bufs=3)

    # ---------------- x + wv loads, staged for earliest v-proj ------------
    xp, xbf = [], []
    for kt in range(KD):
        t = xpp.tile([128, S + 2], F32, name=f"xp{kt}", tag=f"xp{kt}")
        xp.append(t)
        b = xbfp.tile([128, S], BF16, name=f"xbf{kt}", tag=f"xbf{kt}")
        xbf.append(b)
    wv = [wvp.tile([128, D], BF16, name=f"wv{kt}", tag=f"wv{kt}")
          for kt in range(KD)]
    # wv loads dispatch on the (startup-idle) ACT engine's HW DMA queue so
    # they don't serialize behind the x staging on the sync engine.
    for kt in range(KD):
        nc.scalar.dma_start(wv[kt][:], io["wv"][kt * 128:(kt + 1) * 128, :])
    # x arrives fp16; stage through a rotating SBUF tile, then fan out to
    # the bf16 matmul copy (DVE) and the fp32 conv/LN copy (DVE).
    def stage_x(kt, c0, c1):
        xh = xhp.tile([128, 512], F16, name="xh", tag="xh",
                      bufs=3)[:, :c1 - c0]
        nc.sync.dma_start(xh, io["xT"][kt * 128:(kt + 1) * 128, c0:c1])
        nc.vector.tensor_copy(xbf[kt][:, c0:c1], xh)
        nc.vector.tensor_copy(xp[kt][:, 1 + c0:1 + c1], xh)

    # first seq tile of x (enables v-proj st=0), then the rest
    for kt in range(KD):
        stage_x(kt, 0, 128)
    for kt in range(KD):
        stage_x(kt, 128, SQ)
    for kt in range(KD):
        stage_x(kt, SQ, S)
        nc.vector.memset(xp[kt][:, 0:1], 0.0)
        nc.vector.memset(xp[kt][:, S + 1:S + 2], 0.0)

    # ---------------- v projection (x stationary, wv moving) --------------
    v_t = [vtp.tile([128, D], BF16, name=f"vt{st}", tag=f"vt{st}")
           for st in range(NKT)]
    for st in range(NKT):
        for ec in range(NVC):
            ps = ps_tile()
            for kt in range(KD):
                nc.tensor.matmul(ps[:, 0:VC],
                                 xbf[kt][:, st * 128:(st + 1) * 128],
                                 wv[kt][:, ec * VC:(ec + 1) * VC],
                                 start=(kt == 0), stop=False)
            # bv bias via K=1 ones-row matmul into the same accumulation
            nc.tensor.matmul(ps[:, 0:VC], ones_bf[0:1, 0:128],
                             bvr_bf[0:1, ec * VC:(ec + 1) * VC],
                             start=False, stop=True)
            nc.vector.tensor_copy(v_t[st][:, ec * VC:(ec + 1) * VC],
                                  ps[:, 0:VC])
    xhp.release()
    wvp.release()

    # ---------------- conv residual on GPSIMD: racc = 0.3*conv ------------
    # (x itself is NOT accumulated here; it is added back in fp32 at LN
    #  time.  racc is bf16: it only ever holds conv + attention output.)
    racc = [rp.tile([128, S], BF16, name=f"ra{kt}", tag=f"ra{kt}")
            for kt in range(KD)]
    for kt in range(KD):
        for qc in range(NQ):
            o = qc * SQ
            xl, xc, xr = (xp[kt][:, o:o + SQ], xp[kt][:, o + 1:o + SQ + 1],
                          xp[kt][:, o + 2:o + SQ + 2])
            t1 = tmpp.tile([128, SQ], F32, name="lnt", tag="lnt", bufs=4)
            nc.vector.tensor_scalar(t1[:], xl, ct["cw0"][:, kt:kt + 1], None,
                                    op0=OP.mult)
            t2 = tmpp.tile([128, SQ], F32, name="lnt", tag="lnt", bufs=4)
            nc.vector.scalar_tensor_tensor(t2[:], xc, ct["cw1"][:, kt:kt + 1],
                                           t1[:], op0=OP.mult, op1=OP.add)
            nc.vector.scalar_tensor_tensor(racc[kt][:, qs(qc)], xr,
                                           ct["cw2"][:, kt:kt + 1], t2[:],
                                           op0=OP.mult, op1=OP.add)

    # ---------------- q/k projection (paired psum: both qc at once) -------
    q_fm = [qkp.tile([128, S], BF16, name=f"q{m}", tag=f"q{m}")
            for m in range(KD)]
    k_fm = [qkp.tile([128, S], BF16, name=f"k{m}", tag=f"k{m}")
            for m in range(KD)]

    def wd_tile():
        return wd.tile([128, D], BF16, name="wd", tag="wd", bufs=3)

    def qk_proj(m):
        for wname, bias, dst in (("wqs", ct["bq"], q_fm),
                                 ("wks", ct["bk"], k_fm)):
            wt = wd_tile()
            nc.sync.dma_start(wt[:], io[wname][:, m * D:(m + 1) * D])
            pp = pp_tile()
            for qc in range(NQ):
                for kt in range(KD):
                    nc.tensor.matmul(pp[:, qc, :],
                                     wt[:, kt * 128:(kt + 1) * 128],
                                     xbf[kt][:, qs(qc)],
                                     start=(kt == 0), stop=(kt == KD - 1))
            nc.scalar.activation(dst[m][:], pp[:], AF.Identity,
                                 bias=bias[:, m:m + 1])

    # ---------------- attention ----------------
    inv_sqrt_dk = 1.0 / math.sqrt(DK)
    _ao = []

    def attn(hp, qc):
        attnout = _ao
        h0, h1 = 2 * hp, 2 * hp + 1
        eps_ = []
        for kt in range(NKT):
            pp = pp_tile()
            nc.tensor.matmul(pp[:, 0, :],
                             k_fm[hp][0:64, kt * 128:(kt + 1) * 128],
                             q_fm[hp][0:64, qs(qc)], start=True, stop=True)
            nc.tensor.matmul(pp[:, 1, :],
                             k_fm[hp][64:128, kt * 128:(kt + 1) * 128],
                             q_fm[hp][64:128, qs(qc)], start=True, stop=True)
            ep = expp.tile([128, 2, SQ], BF16, name="exp", tag="exp", bufs=9)
            nc.scalar.activation(ep[:], pp[:], AF.Exp, scale=inv_sqrt_dk)
            eps_.append(ep)
        den_t, U_t = ps_tile(), ps_tile()
        den, U = den_t[:, :], U_t[:, :]
        for kt in range(NKT):
            nc.tensor.matmul(den[0:64, :], ones_bf[:, 0:64], eps_[kt][:, 0, :],
                             start=(kt == 0), stop=(kt == NKT - 1))
        for kt in range(NKT):
            nc.tensor.matmul(den[64:128, :], ones_bf[:, 0:64],
                             eps_[kt][:, 1, :],
                             start=(kt == 0), stop=(kt == NKT - 1))
        for kt in range(NKT):
            nc.tensor.matmul(U[0:64, :], v_t[kt][:, h0 * DK:(h0 + 1) * DK],
                             eps_[kt][:, 0, :],
                             start=(kt == 0), stop=(kt == NKT - 1))
        for kt in range(NKT):
            nc.tensor.matmul(U[64:128, :], v_t[kt][:, h1 * DK:(h1 + 1) * DK],
                             eps_[kt][:, 1, :],
                             start=(kt == 0), stop=(kt == NKT - 1))
        recb = smallp.tile([128, SQ], F32, name="recb", tag="recb", bufs=2)
        nc.vector.reciprocal(recb[:], den)
        nc.vector.tensor_tensor(attnout[hp][:, qs(qc)], U, recb[:],
                                op=OP.mult)

    # ---------------- Wo accumulate into racc ----------------
    def wo_step(m, qc):
        attnout = _ao
        wt = wd_tile()
        nc.sync.dma_start(wt[:], io["wos"][:, m * D:(m + 1) * D])
        ps = ps_tile()
        for kt in range(KD):
            nc.tensor.matmul(ps[:], wt[:, kt * 128:(kt + 1) * 128],
                             attnout[kt][:, qs(qc)],
                             start=(kt == 0), stop=(kt == KD - 1))
        nc.vector.scalar_tensor_tensor(racc[m][:, qs(qc)], ps[:],
                                       ct["bocb"][:, m:m + 1],
                                       racc[m][:, qs(qc)],
                                       op0=OP.add, op1=OP.add)

    # ---------------- layernorm helpers (broadcast stats) ----------------
    def vtile(tag, dt=F32):
        return vecp.tile([128, SQ], dt, name=tag, tag=tag, bufs=1)

    def ln_stats(cast_fn):
        """Returns psum pair (ssum at [:,0,:], ssq at [:,1,:])."""
        sp = pp_tile()
        for kt in range(KD):
            rbf = cast_fn(kt)
            sq_t = sqp.tile([128, SQ], BF16, name="sq", tag="sq", bufs=2)
            nc.vector.tensor_tensor(sq_t[:], rbf[:], rbf[:], op=OP.mult)
            st_, sp_ = (kt == 0), (kt == KD - 1)
            nc.tensor.matmul(sp[:, 0, :], ones_bf[:, 0:128], rbf[:],
                             start=st_, stop=sp_)
            nc.tensor.matmul(sp[:, 1, :], ones_bf[:, 0:128], sq_t[:],
                             start=st_, stop=sp_)
        return sp

    def ln_finish(sp, c0=0, c1=None):
        """mu, rstd [128, c1-c0] from the stats pair (Newton rsqrt on DVE)."""
        c1 = SQ if c1 is None else c1
        mu_t, var_t, rst_t = vtile("vmu"), vtile("vvar"), vtile("vrst")
        mu, var, rst = mu_t[:, c0:c1], var_t[:, c0:c1], rst_t[:, c0:c1]
        nc.vector.tensor_scalar_mul(mu, sp[:, 0, c0:c1], 1.0 / D)
        nc.vector.tensor_tensor(var, mu, mu, op=OP.mult)
        nc.vector.scalar_tensor_tensor(var, sp[:, 1, c0:c1], 1.0 / D, var,
                                       op0=OP.mult, op1=OP.subtract)
        nc.vector.tensor_scalar_add(var, var, EPS)
        rst_i = rst_t[:].bitcast(I32)[:, c0:c1]
        var_i = var_t[:].bitcast(I32)[:, c0:c1]
        nc.vector.tensor_scalar(rst_i, var_i, 1, None,
                                op0=OP.arith_shift_right)
        nc.vector.tensor_scalar(rst_i, rst_i, -1, MAGIC, op0=OP.mult,
                                op1=OP.add)
        for _ in range(2):
            t = vtile("vnt")[:, c0:c1]
            nc.vector.tensor_tensor(t, rst, rst, op=OP.mult)
            nc.vector.tensor_tensor(t, t, var, op=OP.mult)
            nc.vector.tensor_scalar(t, t, -0.5, 1.5, op0=OP.mult,
                                    op1=OP.add)
            nc.vector.tensor_tensor(rst, rst, t, op=OP.mult)
        return mu, rst

    def ln_norm_simple(src_fn, mu, rst, write_out, c0=0, c1=None):
        c1 = SQ if c1 is None else c1
        for kt in range(KD):
            # alternate DVE/Pool so the (otherwise serial) normalize chain
            # runs on two engines; Pool is idle in the n2 tail
            eng = nc.vector if kt % 2 == 0 else nc.gpsimd
            t1 = tmpp.tile([128, SQ], F32, name="lnt", tag="lnt", bufs=4)
            eng.tensor_tensor(t1[:, c0:c1], src_fn(kt), mu,
                              op=OP.subtract)
            eng.tensor_tensor(t1[:, c0:c1], t1[:, c0:c1], rst,
                              op=OP.mult)
            write_out(kt, t1)

    # ---------------- LN chains ----------------
    def lna_r2(qc):
        """lna on (x + racc); then r2 = x + lna_out accumulated into xp."""
        def cast_lna(kt):
            rbf = sqp.tile([128, SQ], BF16, name="rbf", tag="rbf", bufs=2)
            nc.gpsimd.tensor_tensor(
                rbf[:], xp[kt][:, 1 + qc * SQ:1 + (qc + 1) * SQ],
                racc[kt][:, qs(qc)], op=OP.add)
            return rbf

        sp = ln_stats(cast_lna)
        mu, rst = ln_finish(sp)
        for kt in range(KD):
            xs = xp[kt][:, 1 + qc * SQ:1 + (qc + 1) * SQ]
            t1 = tmpp.tile([128, SQ], F32, name="lnt", tag="lnt", bufs=4)
            nc.vector.tensor_tensor(t1[:], xs, racc[kt][:, qs(qc)], op=OP.add)
            nc.vector.tensor_tensor(t1[:], t1[:], mu[:], op=OP.subtract)
            nc.vector.tensor_tensor(t1[:], t1[:], rst[:], op=OP.mult)
            t2 = tmpp.tile([128, SQ], F32, name="lnt", tag="lnt", bufs=4)
            nc.scalar.activation(t2[:], t1[:], AF.Identity,
                                 bias=ct["lnab"][:, kt:kt + 1],
                                 scale=ct["lnag"][:, kt:kt + 1])
            nc.gpsimd.tensor_tensor(xs, xs, t2[:], op=OP.add)

    h_holder = []

    def n1_chain(qc):
        h_bf = h_holder

        def cast_n1(kt):
            rbf = sqp.tile([128, SQ], BF16, name="rbf", tag="rbf", bufs=2)
            nc.gpsimd.tensor_copy(rbf[:],
                                  xp[kt][:, 1 + qc * SQ:1 + (qc + 1) * SQ])
            return rbf

        sp = ln_stats(cast_n1)
        mu, rst = ln_finish(sp)

        def w_n1(kt, t1):
            nc.scalar.activation(h_bf[kt][:, qs(qc)], t1[:], AF.Identity,
                                 bias=ct["n1b"][:, kt:kt + 1],
                                 scale=ct["n1g"][:, kt:kt + 1])

        ln_norm_simple(
            lambda kt: xp[kt][:, 1 + qc * SQ:1 + (qc + 1) * SQ], mu, rst, w_n1)

    # ========================================================================
    # schedule
    # ========================================================================
    for m in range(KD):
        qk_proj(m)
    _ao.extend(aop.tile([128, S], BF16, name=f"ao{m}", tag=f"ao{m}")
               for m in range(KD))
    h_holder.extend(xbf)    # reuse the dead x-bf16 tiles as h storage
    for hp in range(HP):
        attn(hp, 0)
    for m in range(KD):
        wo_step(m, 0)
    lna_r2(0)               # chains run on DVE/Pool under attn(qc1) below
    for hp in range(HP // 2):
        attn(hp, 1)
    n1_chain(0)
    for hp in range(HP // 2, HP):
        attn(hp, 1)
    expp.release()
    qkp.release()
    vtp.release()

    for m in range(KD):
        wo_step(m, 1)
    aop.release()
    lna_r2(1)               # chain overlaps FFN-W1(qc0)

    f1p = pool(name="ffn1", bufs=1, side="right")

    def ffn_w1(qc, mid=None):
        f1t = []
        for m in range(KF):
            if m == 8 and mid is not None:
                mid()
            wt = wd_tile()
            nc.sync.dma_start(wt[:], io["w1s"][:, m * D:(m + 1) * D])
            ps = ps_tile()
            for kt in range(KD):
                nc.tensor.matmul(ps[:], wt[:, kt * 128:(kt + 1) * 128],
                                 h_holder[kt][:, qs(qc)],
                                 start=(kt == 0), stop=(kt == KD - 1))
            t = f1p.tile([128, SQ], BF16, name=f"f1_{m}", tag=f"f1_{m}")
            nc.scalar.activation(t[:], ps[:], AF.Gelu,
                                 bias=ct["b1t"][:, m:m + 1])
            f1t.append(t)
        return f1t

    f1_qc0 = ffn_w1(0, mid=lambda: n1_chain(1))

    wf = pool(name="wf", bufs=1)
    fop = pool(name="fout", bufs=1)
    f2p = pool(name="ffn2", bufs=1, side="right")
    fout = [fop.tile([128, SQ], F32, name=f"fo{m}", tag=f"fo{m}")
            for m in range(KD)]

    def wf_tile():
        """Half of a W2/W3 contraction block: [128, F//2]."""
        return wf.tile([128, F // 2], BF16, name="wf", tag="wf", bufs=4)

    def big_mm(ps, wname, m, rhs_tiles):
        """Accumulate over KF k-tiles streaming weights in two half tiles."""
        for h in range(2):
            wt = wf_tile()
            nc.sync.dma_start(wt[:], io[wname][:, m * F + h * (F // 2):
                                               m * F + (h + 1) * (F // 2)])
            for j in range(KF // 2):
                kt = h * (KF // 2) + j
                nc.tensor.matmul(ps, wt[:, j * 128:(j + 1) * 128],
                                 rhs_tiles[kt][:],
                                 start=(kt == 0), stop=(kt == KF - 1))

    def ffn_rest(qc, f1t, mid=None):
        h_bf = h_holder
        f2t = []
        for m in range(KF):
            if m == 8 and mid is not None:
                mid()
            ps = ps_tile()
            big_mm(ps[:], "w2s", m, f1t)
            t = f2p.tile([128, SQ], BF16, name=f"f2_{m}", tag=f"f2_{m}")
            nc.scalar.activation(t[:], ps[:], AF.Gelu,
                                 bias=ct["b2t"][:, m:m + 1])
            f2t.append(t)
        # W3 + gate, with incremental n2 statistics per output tile
        n2sp = pp_tile()
        for m in range(KD):
            wtg = wd_tile()
            nc.sync.dma_start(wtg[:], io["wgs"][:, m * D:(m + 1) * D])
            gp = pp_tile()
            psg, ps3 = gp[:, 0, :], gp[:, 1, :]
            for kt in range(KD):
                nc.tensor.matmul(psg, wtg[:, kt * 128:(kt + 1) * 128],
                                 h_bf[kt][:, qs(qc)],
                                 start=(kt == 0), stop=(kt == KD - 1))
            gat = tmpp.tile([128, SQ], BF16, name="gat", tag="gat", bufs=2)
            nc.scalar.activation(gat[:], psg, AF.Sigmoid,
                                 bias=ct["bgt"][:, m:m + 1])
            big_mm(ps3, "w3s", m, f2t)
            t = tmpp.tile([128, SQ], F32, name="f3t", tag="f3t", bufs=2)
            nc.vector.scalar_tensor_tensor(t[:], ps3, ct["b3t"][:, m:m + 1],
                                           gat[:], op0=OP.add, op1=OP.mult)
            nc.vector.tensor_tensor(fout[m][:], t[:],
                                    h_bf[m][:, qs(qc)], op=OP.add)
            # incremental n2 stats for this feature tile
            rbf = sqp.tile([128, SQ], BF16, name="rbf", tag="rbf", bufs=2)
            nc.gpsimd.tensor_copy(rbf[:], fout[m][:])
            sq_t = sqp.tile([128, SQ], BF16, name="sq", tag="sq", bufs=2)
            nc.scalar.square(sq_t[:], rbf[:])
            st_, sp_ = (m == 0), (m == KD - 1)
            nc.tensor.matmul(n2sp[:, 0, :], ones_bf[:, 0:128], rbf[:],
                             start=st_, stop=sp_)
            nc.tensor.matmul(n2sp[:, 1, :], ones_bf[:, 0:128], sq_t[:],
                             start=st_, stop=sp_)
        return n2sp

    def emit_n2(qc, n2sp):
        mu, rst = ln_finish(n2sp)

        def w_n2(kt, t1):
            stg = stgp.tile([128, SQ], F16, name="stg", tag="stg", bufs=2)
            nc.scalar.activation(stg[:], t1[:], AF.Identity,
                                 bias=ct["n2b"][:, kt:kt + 1],
                                 scale=ct["n2g"][:, kt:kt + 1])
            nc.sync.dma_start(io["outT"][kt * 128:(kt + 1) * 128, qs(qc)],
                              stg[:])

        ln_norm_simple(lambda kt: fout[kt][:], mu, rst, w_n2)

    n2sp0 = ffn_rest(0, f1_qc0)
    emit_n2(0, n2sp0)
    f1_qc1 = ffn_w1(1)
    n2sp1 = ffn_rest(1, f1_qc1)
    emit_n2(1, n2sp1)

    f2p.release()
    f1p.release()
    fop.release()
    wf.release()
    xbfp.release()
    xpp.release()
    rp.release()
    wd.release()
    stgp.release()
    smallp.release()
    sqp.release()
    vecp.release()
    tmpp.release()
    consts.release()
    psum.release()


# ------------------------------------------------------------------
# host side
# ------------------------------------------------------------------

def _shuffle_w(w):
    """[K, E] -> [128, (E//128)*K] bf16 so that slice [:, m*K:(m+1)*K]
    viewed as [128, K//128, 128] gives lhsT tiles w[kt*128+p, m*128+c]."""
    K, E = w.shape
    r = np.asarray(w).reshape(K // 128, 128, E // 128, 128).transpose(1, 2, 0, 3)
    return np.ascontiguousarray(r.reshape(128, (E // 128) * K)).astype(
        ml_dtypes.bfloat16)


def _ptable(b):
    """[E] -> [128, E//128] per-partition scalar table."""
    return np.ascontiguousarray(np.asarray(b, np.float32).reshape(-1, 128).T)


def _declare_io(nc, cfg, consts=None):
    """xT/outT are per-call I/O; every weight tensor is embedded in the
    NEFF as a Const DRAM tensor (DMA'd to HBM once at model-load time) so
    repeated executions only ship the activation tensors."""
    S, D, F, KD, KF = cfg.S, cfg.D, cfg.F, cfg.KD, cfg.KF
    io = {}

    def inp(name, shape, dt):
        if consts is not None and name in consts:
            arr = np.ascontiguousarray(consts[name])
            assert list(arr.shape) == list(shape), (name, arr.shape, shape)
            io[name] = nc.inline_tensor(arr, name=name).ap()
        else:
            io[name] = nc.dram_tensor(name, shape, dt, kind="ExternalInput").ap()

    inp("xT", [D, S], F16)
    inp("wqs", [128, KD * D], BF16)
    inp("wks", [128, KD * D], BF16)
    inp("wv", [D, D], BF16)
    inp("wos", [128, KD * D], BF16)
    inp("w1s", [128, KF * D], BF16)
    inp("w2s", [128, KF * F], BF16)
    inp("wgs", [128, KD * D], BF16)
    inp("w3s", [128, KD * F], BF16)
    inp("cpk", [128, 16 * KD + 2 * KF], F32)
    inp("bvr", [1, D], BF16)
    io["outT"] = nc.dram_tensor("outT", [D, S], F16, kind="ExternalOutput").ap()
    return io


def build_shared_inputs(inputs, cfg):
    """Everything except xT (identical across cores)."""
    f32 = np.float32
    g = {k: np.asarray(v) for k, v in inputs.items()}
    ptabs = [
        _ptable(g["bq"]), _ptable(g["bk"]),
        _ptable(np.asarray(g["bo"], f32) + 0.3 * np.asarray(g["conv_b"], f32)),
        _ptable(0.3 * np.asarray(g["conv_w"], f32)[:, 0]),
        _ptable(0.3 * np.asarray(g["conv_w"], f32)[:, 1]),
        _ptable(0.3 * np.asarray(g["conv_w"], f32)[:, 2]),
        _ptable(g["b3"]), _ptable(g["bg"]),
        _ptable(g["lna_g"]), _ptable(g["lna_b"]),
        _ptable(g["n1_g"]), _ptable(g["n1_b"]),
        _ptable(g["n2_g"]), _ptable(g["n2_b"]),
        _ptable(np.asarray(g["lna_b"], f32)
                / np.where(np.abs(np.asarray(g["lna_g"], f32)) < 1e-20,
                           1.0, np.asarray(g["lna_g"], f32))),
        _ptable(np.asarray(g["n1_b"], f32)
                / np.where(np.abs(np.asarray(g["n1_g"], f32)) < 1e-20,
                           1.0, np.asarray(g["n1_g"], f32))),
        _ptable(g["b1"]), _ptable(g["b2"]),
    ]
    sh = {
        "wqs": _shuffle_w(g["Wq"]), "wks": _shuffle_w(g["Wk"]),
        "wv": np.ascontiguousarray(g["Wv"]).astype(ml_dtypes.bfloat16),
        "wos": _shuffle_w(g["Wo"]), "w1s": _shuffle_w(g["W1"]),
        "w2s": _shuffle_w(g["W2"]), "w3s": _shuffle_w(g["W3"]),
        "wgs": _shuffle_w(g["Wg"]),
        "cpk": np.ascontiguousarray(np.concatenate(ptabs, axis=1)),
        "bvr": np.ascontiguousarray(
            np.asarray(g["bv"], f32).reshape(1, cfg.D)).astype(
                ml_dtypes.bfloat16),
    }
    return sh


_CACHE = {}


def _weights_fingerprint(inputs):
    """Content hash of every non-x input. The NEFF embeds the weights, so
    a changed weight set must rebuild (and recompile) the kernel."""
    import hashlib
    h = hashlib.blake2b(digest_size=16)
    for k in sorted(inputs):
        if k == "x":
            continue
        a = np.ascontiguousarray(inputs[k])
        h.update(k.encode())
        h.update(str(a.shape).encode())
        h.update(str(a.dtype).encode())
        h.update(a.tobytes())
    return h.hexdigest()


def _get_nc(inputs):
    key = _weights_fingerprint(inputs)
    ent = _CACHE.get("nc")
    if ent is None or ent[0] != key:
        shared = build_shared_inputs(inputs, FULL)
        nc = bass.Bass("TRN2", target_bir_lowering=False, debug=False)
        io = _declare_io(nc, FULL, consts=shared)
        with _TC(nc) as tc:
            emit(tc, FULL, io)
        _CACHE["nc"] = (key, nc)
    return _CACHE["nc"][1]


def _get_exec(inputs):
    """Persistent jitted executor: one bass_exec custom call shard_mapped
    over the 8 cores.  Rebuilt only when the weight set changes."""
    import jax
    from jax.sharding import Mesh, PartitionSpec
    from jax.experimental.shard_map import shard_map
    from concourse import bass2jax

    key = _weights_fingerprint(inputs)
    ent = _CACHE.get("exec")
    if ent is not None and ent[0] == key:
        return ent[1]

    bass2jax.install_neuronx_cc_hook()
    nc = _get_nc(inputs)
    pid = nc.partition_id_tensor.name if nc.partition_id_tensor else None
    in_names, out_names, out_avals = [], [], []
    for alloc in nc.m.functions[0].allocations:
        if not isinstance(alloc, mybir.MemoryLocationSet):
            continue
        name = alloc.memorylocations[0].name
        if alloc.kind == "ExternalInput":
            if name != pid:
                in_names.append(name)
        elif alloc.kind == "ExternalOutput":
            out_names.append(name)
            out_avals.append(jax.core.ShapedArray(
                tuple(alloc.tensor_shape), mybir.dt.np(alloc.dtype)))
    all_names = in_names + ([pid] if pid is not None else [])
    assert in_names == ["xT"] and out_names == ["outT"]

    def _body(*args):
        operands = list(args)
        if pid is not None:
            operands.append(bass2jax.partition_id_tensor())
        return tuple(bass2jax._bass_exec_p.bind(
            *operands, out_avals=tuple(out_avals), in_names=tuple(all_names),
            out_names=tuple(out_names), lowering_input_output_aliases=(),
            sim_require_finite=True, sim_require_nnan=True, nc=nc))

    mesh = Mesh(np.asarray(jax.devices()[:N_CORES]), ("core",))
    sharded = jax.jit(
        shard_map(_body, mesh=mesh, in_specs=(PartitionSpec("core"),),
                  out_specs=(PartitionSpec("core"),), check_rep=False),
        keep_unused=True)
    _CACHE["exec"] = (key, sharded)
    return sharded


def kernel(**inputs):
    import jax

    cfg = FULL
    sharded = _get_exec(inputs)
    x = np.asarray(inputs["x"], dtype=np.float32)
    B = x.shape[0]
    assert B == N_CORES
    xc = np.concatenate([x[b].T.astype(np.float16, order="C")
                         for b in range(B)], axis=0)
    out = sharded(jax.device_put(xc))
    o0 = np.asarray(out[0]).astype(np.float32).reshape(B, cfg.D, cfg.S)
    return np.ascontiguousarray(o0.transpose(0, 2, 1))



# revision 19
# speedup vs baseline: 1.0358x; 1.0125x over previous
"""Trainium2 Bass kernel for an enhanced transformer layer.

Strategy: data-parallel over batch (B=8 -> one batch element per NeuronCore,
no collectives).  Activations are kept feature-major ([D, S] with the
contraction dim on partitions) so every linear layer consumes weights in
natural [K, E] layout as the stationary operand.  Matmuls run in bf16 with
fp32 PSUM accumulation.

I/O strategy: all weight tensors are embedded in the NEFF as Const DRAM
tensors (DMA'd to HBM once at model-load time), so per-execution bindings
are just xT (fp16 in) and outT (fp16 out) -- 2 MB each per core.  The NEFF
is rebuilt (keyed on a content hash of the weights) if kernel() is called
with a different weight set.  fp16 transport adds ~2^-11 relative
quantization noise on x and out, negligible against the bf16 matmul noise.

Schedule: the layer is software-pipelined over the two S/2 column chunks
(qc) so LayerNorm / softmax vector work overlaps matmul streams:
  v-proj (staged x arrival) -> qk-proj -> attn(qc0) -> [attn(hp, qc1) |
  Wo(qc0)] -> lna(qc0) | Wo(qc1).a -> n1(qc0) | Wo(qc1).b ->
  FFN-W1(qc0) | lna(qc1) -> FFN-W2(qc0) | n1(qc1) -> FFN-W3/gate(qc0) with
  incremental n2(qc0) stats -> n2(qc0) | FFN(qc1) -> n2(qc1).
LayerNorm statistics use M=128 ones-matmuls that produce row-broadcast sums
directly (no scalar-row chains, no separate broadcast matmuls), and
1/sqrt(var) is computed on DVE via Newton iteration so the ACT engine never
loads the sqrt table (a table-set switch costs ~2.7us).  Softmax
denominators are likewise accumulated as broadcast ones-matmuls; exp is
evaluated on paired PSUM banks ([128, 2*SQ] per instruction) to halve ACT
dispatch count in the attention phase.  The depthwise-conv residual and
h=x+attn adds run on GPSIMD (Pool).  The attention residual accumulator
(conv + Wo output, without x) is kept in bf16; x is added back in fp32 at
LN time.  All small constants ship in one packed DMA (dma_start dispatch
costs ~0.65us each on the sync engine).
"""

import math

import numpy as np
import ml_dtypes

import concourse.bass as bass
import concourse.tile as tile
from concourse import mybir
from concourse.alu_op_type import AluOpType
from bass_rust import ScopedClock

F32 = mybir.dt.float32
I32 = mybir.dt.int32
BF16 = mybir.dt.bfloat16
F16 = mybir.dt.float16
AF = mybir.ActivationFunctionType
OP = AluOpType

EPS = 1e-5
N_CORES = 8
MAGIC = 0x5F3759DF


class CFG:
    def __init__(self, S=1024, D=1024, F=4096, H=16):
        self.S, self.D, self.F, self.H = S, D, F, H
        self.DK = D // H              # head dim (must be 64)
        self.KD = D // 128            # feature tiles of model dim
        self.KF = F // 128            # feature tiles of ffn dim
        self.SQ = min(512, S)         # moving-dim chunk
        self.NQ = S // self.SQ
        self.NKT = S // 128           # key/sequence tiles
        self.VC = min(512, D)         # v-projection output chunk
        self.NVC = D // self.VC
        assert self.DK == 64 and H % 2 == 0


FULL = CFG()


def _split_excess_waits(nc, max_waits=1):
    """Walrus in this container rejects >2 sync waits per instruction.
    Hoist excess waits onto same-engine nops inserted just before."""
    cnt = 0
    for fn in nc.m.functions:
        for bb in fn.blocks:
            insts = list(bb.instructions)
            out = []
            for inst in insts:
                si = inst.sync_info
                waits = list(si.on_wait) if si and si.on_wait else []
                if len(waits) > max_waits:
                    extra = waits[:-max_waits]
                    si.on_wait = waits[-max_waits:]
                    for i in range(0, len(extra), max_waits):
                        cnt += 1
                        out.append(mybir.InstNoOp(
                            name=f"waitsplit{cnt}_{inst.name}",
                            engine=inst.engine, ins=[], outs=[],
                            sync_info=mybir.SyncInfo(
                                on_wait=extra[i:i + max_waits], on_update=[]),
                        ))
                out.append(inst)
            if cnt:
                bb.instructions = out
    return cnt


class _TC(tile.TileContext):
    """TileContext whose exit drain spreads semaphore waits over several
    sync-engine nops -- this container's walrus rejects >2 sync waits on a
    single CTRL instruction."""

    def __exit__(self, *a):
        r = super().__exit__(*a)
        _split_excess_waits(self.nc)
        return r

    def _drain_and_barrier(self, tick_clock, wait_clock):
        nc = self.nc
        drain_inst = nc.sync.drain()
        wait_clock.add_sem_waits(
            drain_inst.ins, ScopedClock({None: tick_clock.global_clock})
        )
        si = drain_inst.ins.sync_info
        waits = list(si.on_wait) if si and si.on_wait else []
        if len(waits) > 1:
            si.on_wait = waits[:1]
            for w in waits[1:]:
                nop = nc.sync.nop(nofuse=True)
                nsi = nop.ins.sync_info
                if nsi is None:
                    nop.ins.sync_info = mybir.SyncInfo(on_wait=[w], on_update=[])
                else:
                    nsi.on_wait = [w]
        nc.all_engine_barrier()
        popped = nc._tile_sem_poison_stack.pop()
        assert popped is self._sem_poison
        nc.clear_and_free_semaphores(list(self.sems.allocated().values()))
        nc.all_engine_barrier()


def emit(tc, cfg, io):
    nc = tc.nc
    S, D, F, H = cfg.S, cfg.D, cfg.F, cfg.H
    DK, KD, KF = cfg.DK, cfg.KD, cfg.KF
    SQ, NQ, NKT = cfg.SQ, cfg.NQ, cfg.NKT
    VC, NVC = cfg.VC, cfg.NVC
    HP = H // 2

    def qs(qc):
        return slice(qc * SQ, (qc + 1) * SQ)

    pool = tc.alloc_tile_pool

    # ======== pools: alloc order is EXACT reverse death order per side ====
    consts = pool(name="consts", bufs=1)
    tmpp = pool(name="tmp", bufs=1)
    vecp = pool(name="vec", bufs=1)
    sqp = pool(name="sq", bufs=1)
    smallp = pool(name="small", bufs=1)
    stgp = pool(name="stg", bufs=1)
    wd = pool(name="wd", bufs=1)
    rp = pool(name="racc", bufs=1)           # bf16 conv+Wo residual
    xpp = pool(name="xp", bufs=1)
    xbfp = pool(name="xbf", bufs=1)          # x bf16, then reused as h
    vtp = pool(name="vt", bufs=1, side="right")
    wvp = pool(name="wv", bufs=1, side="right")
    xhp = pool(name="xh", bufs=1, side="right")  # fp16 x staging
    aop = pool(name="aop", bufs=1)
    qkp = pool(name="qk", bufs=1)
    expp = pool(name="expT", bufs=1)
    psum = pool(name="ps", bufs=1, space="PSUM")

    # ---------------- constants (one packed DMA) ----------------
    cpk = consts.tile([128, 16 * KD + 2 * KF], F32, name="cpk", tag="cpk")
    ct = {}
    off = 0
    for cname in ("bq", "bk", "bocb", "cw0", "cw1", "cw2", "b3t", "bgt",
                  "lnag", "lnab", "n1g", "n1b", "n2g", "n2b", "bdla", "bdn1"):
        ct[cname] = cpk[:, off:off + KD]
        off += KD
    for cname in ("b1t", "b2t"):
        ct[cname] = cpk[:, off:off + KF]
        off += KF
    ones_bf = consts.tile([128, 128], BF16, name="onesbf", tag="onesbf")
    nc.vector.memset(ones_bf[:], 1.0)
    bvr_bf = consts.tile([1, D], BF16, name="bvrbf", tag="bvrbf")
    nc.scalar.dma_start(bvr_bf[:], io["bvr"][:, :])
    nc.scalar.dma_start(cpk[:], io["cpk"][:, :])

    def ps_tile():
        return psum.tile([128, SQ], F32, name="ps", tag="ps", bufs=2)

    def pp_tile():
        return psum.tile([128, 2, SQ], F32, name="pp", tag="pp", bufs=3)

    # ---------------- x + wv loads, staged for earliest v-proj ------------
    xp, xbf = [], []
    for kt in range(KD):
        t = xpp.tile([128, S + 2], F32, name=f"xp{kt}", tag=f"xp{kt}")
        xp.append(t)
        b = xbfp.tile([128, S], BF16, name=f"xbf{kt}", tag=f"xbf{kt}")
        xbf.append(b)
    wv = [wvp.tile([128, D], BF16, name=f"wv{kt}", tag=f"wv{kt}")
          for kt in range(KD)]
    # wv loads dispatch on the (startup-idle) ACT engine's HW DMA queue so
    # they don't serialize behind the x staging on the sync engine.
    for kt in range(KD):
        nc.scalar.dma_start(wv[kt][:], io["wv"][kt * 128:(kt + 1) * 128, :])
    # x arrives fp16; stage through a rotating SBUF tile, then fan out to
    # the bf16 matmul copy (DVE) and the fp32 conv/LN copy (DVE).
    def stage_x(kt, c0, c1):
        xh = xhp.tile([128, 512], F16, name="xh", tag="xh",
                      bufs=3)[:, :c1 - c0]
        nc.sync.dma_start(xh, io["xT"][kt * 128:(kt + 1) * 128, c0:c1])
        nc.vector.tensor_copy(xbf[kt][:, c0:c1], xh)
        nc.vector.tensor_copy(xp[kt][:, 1 + c0:1 + c1], xh)

    # first seq tile of x (enables v-proj st=0), then the rest
    for kt in range(KD):
        stage_x(kt, 0, 128)
    for kt in range(KD):
        stage_x(kt, 128, SQ)
    for kt in range(KD):
        stage_x(kt, SQ, S)
        nc.vector.memset(xp[kt][:, 0:1], 0.0)
        nc.vector.memset(xp[kt][:, S + 1:S + 2], 0.0)

    # ---------------- v projection (x stationary, wv moving) --------------
    v_t = [vtp.tile([128, D], BF16, name=f"vt{st}", tag=f"vt{st}")
           for st in range(NKT)]
    for st in range(NKT):
        for ec in range(NVC):
            ps = ps_tile()
            for kt in range(KD):
                nc.tensor.matmul(ps[:, 0:VC],
                                 xbf[kt][:, st * 128:(st + 1) * 128],
                                 wv[kt][:, ec * VC:(ec + 1) * VC],
                                 start=(kt == 0), stop=False)
            # bv bias via K=1 ones-row matmul into the same accumulation
            nc.tensor.matmul(ps[:, 0:VC], ones_bf[0:1, 0:128],
                             bvr_bf[0:1, ec * VC:(ec + 1) * VC],
                             start=False, stop=True)
            nc.vector.tensor_copy(v_t[st][:, ec * VC:(ec + 1) * VC],
                                  ps[:, 0:VC])
    xhp.release()
    wvp.release()

    # ---------------- conv residual on GPSIMD: racc = 0.3*conv ------------
    # (x itself is NOT accumulated here; it is added back in fp32 at LN
    #  time.  racc is bf16: it only ever holds conv + attention output.)
    racc = [rp.tile([128, S], BF16, name=f"ra{kt}", tag=f"ra{kt}")
            for kt in range(KD)]
    for kt in range(KD):
        for qc in range(NQ):
            o = qc * SQ
            xl, xc, xr = (xp[kt][:, o:o + SQ], xp[kt][:, o + 1:o + SQ + 1],
                          xp[kt][:, o + 2:o + SQ + 2])
            t1 = tmpp.tile([128, SQ], F32, name="lnt", tag="lnt", bufs=4)
            nc.vector.tensor_scalar(t1[:], xl, ct["cw0"][:, kt:kt + 1], None,
                                    op0=OP.mult)
            t2 = tmpp.tile([128, SQ], F32, name="lnt", tag="lnt", bufs=4)
            nc.vector.scalar_tensor_tensor(t2[:], xc, ct["cw1"][:, kt:kt + 1],
                                           t1[:], op0=OP.mult, op1=OP.add)
            nc.vector.scalar_tensor_tensor(racc[kt][:, qs(qc)], xr,
                                           ct["cw2"][:, kt:kt + 1], t2[:],
                                           op0=OP.mult, op1=OP.add)

    # ---------------- q/k projection (paired psum: both qc at once) -------
    q_fm = [qkp.tile([128, S], BF16, name=f"q{m}", tag=f"q{m}")
            for m in range(KD)]
    k_fm = [qkp.tile([128, S], BF16, name=f"k{m}", tag=f"k{m}")
            for m in range(KD)]

    def wd_tile():
        return wd.tile([128, D], BF16, name="wd", tag="wd", bufs=3)

    def qk_proj(m):
        for wname, bias, dst in (("wqs", ct["bq"], q_fm),
                                 ("wks", ct["bk"], k_fm)):
            wt = wd_tile()
            nc.sync.dma_start(wt[:], io[wname][:, m * D:(m + 1) * D])
            pp = pp_tile()
            for qc in range(NQ):
                for kt in range(KD):
                    nc.tensor.matmul(pp[:, qc, :],
                                     wt[:, kt * 128:(kt + 1) * 128],
                                     xbf[kt][:, qs(qc)],
                                     start=(kt == 0), stop=(kt == KD - 1))
            nc.scalar.activation(dst[m][:], pp[:], AF.Identity,
                                 bias=bias[:, m:m + 1])

    # ---------------- attention ----------------
    inv_sqrt_dk = 1.0 / math.sqrt(DK)
    _ao = []

    def attn(hp, qc):
        attnout = _ao
        h0, h1 = 2 * hp, 2 * hp + 1
        eps_ = []
        for kt in range(NKT):
            pp = pp_tile()
            nc.tensor.matmul(pp[:, 0, :],
                             k_fm[hp][0:64, kt * 128:(kt + 1) * 128],
                             q_fm[hp][0:64, qs(qc)], start=True, stop=True)
            nc.tensor.matmul(pp[:, 1, :],
                             k_fm[hp][64:128, kt * 128:(kt + 1) * 128],
                             q_fm[hp][64:128, qs(qc)], start=True, stop=True)
            ep = expp.tile([128, 2, SQ], BF16, name="exp", tag="exp", bufs=9)
            nc.scalar.activation(ep[:], pp[:], AF.Exp, scale=inv_sqrt_dk)
            eps_.append(ep)
        den_t, U_t = ps_tile(), ps_tile()
        den, U = den_t[:, :], U_t[:, :]
        for kt in range(NKT):
            nc.tensor.matmul(den[0:64, :], ones_bf[:, 0:64], eps_[kt][:, 0, :],
                             start=(kt == 0), stop=(kt == NKT - 1))
        for kt in range(NKT):
            nc.tensor.matmul(den[64:128, :], ones_bf[:, 0:64],
                             eps_[kt][:, 1, :],
                             start=(kt == 0), stop=(kt == NKT - 1))
        for kt in range(NKT):
            nc.tensor.matmul(U[0:64, :], v_t[kt][:, h0 * DK:(h0 + 1) * DK],
                             eps_[kt][:, 0, :],
                             start=(kt == 0), stop=(kt == NKT - 1))
        for kt in range(NKT):
            nc.tensor.matmul(U[64:128, :], v_t[kt][:, h1 * DK:(h1 + 1) * DK],
                             eps_[kt][:, 1, :],
                             start=(kt == 0), stop=(kt == NKT - 1))
        recb = smallp.tile([128, SQ], F32, name="recb", tag="recb", bufs=2)
        nc.vector.reciprocal(recb[:], den)
        nc.vector.tensor_tensor(attnout[hp][:, qs(qc)], U, recb[:],
                                op=OP.mult)

    # ---------------- Wo accumulate into racc ----------------
    def wo_step(m, qc):
        attnout = _ao
        wt = wd_tile()
        nc.sync.dma_start(wt[:], io["wos"][:, m * D:(m + 1) * D])
        ps = ps_tile()
        for kt in range(KD):
            nc.tensor.matmul(ps[:], wt[:, kt * 128:(kt + 1) * 128],
                             attnout[kt][:, qs(qc)],
                             start=(kt == 0), stop=(kt == KD - 1))
        nc.vector.scalar_tensor_tensor(racc[m][:, qs(qc)], ps[:],
                                       ct["bocb"][:, m:m + 1],
                                       racc[m][:, qs(qc)],
                                       op0=OP.add, op1=OP.add)

    # ---------------- layernorm helpers (broadcast stats) ----------------
    def vtile(tag, dt=F32):
        return vecp.tile([128, SQ], dt, name=tag, tag=tag, bufs=1)

    def ln_stats(cast_fn):
        """Returns psum pair (ssum at [:,0,:], ssq at [:,1,:])."""
        sp = pp_tile()
        for kt in range(KD):
            rbf = cast_fn(kt)
            sq_t = sqp.tile([128, SQ], BF16, name="sq", tag="sq", bufs=2)
            nc.vector.tensor_tensor(sq_t[:], rbf[:], rbf[:], op=OP.mult)
            st_, sp_ = (kt == 0), (kt == KD - 1)
            nc.tensor.matmul(sp[:, 0, :], ones_bf[:, 0:128], rbf[:],
                             start=st_, stop=sp_)
            nc.tensor.matmul(sp[:, 1, :], ones_bf[:, 0:128], sq_t[:],
                             start=st_, stop=sp_)
        return sp

    def ln_finish(sp, c0=0, c1=None):
        """mu, rstd [128, c1-c0] from the stats pair (Newton rsqrt on DVE)."""
        c1 = SQ if c1 is None else c1
        mu_t, var_t, rst_t = vtile("vmu"), vtile("vvar"), vtile("vrst")
        mu, var, rst = mu_t[:, c0:c1], var_t[:, c0:c1], rst_t[:, c0:c1]
        nc.vector.tensor_scalar_mul(mu, sp[:, 0, c0:c1], 1.0 / D)
        nc.vector.tensor_tensor(var, mu, mu, op=OP.mult)
        nc.vector.scalar_tensor_tensor(var, sp[:, 1, c0:c1], 1.0 / D, var,
                                       op0=OP.mult, op1=OP.subtract)
        nc.vector.tensor_scalar_add(var, var, EPS)
        rst_i = rst_t[:].bitcast(I32)[:, c0:c1]
        var_i = var_t[:].bitcast(I32)[:, c0:c1]
        nc.vector.tensor_scalar(rst_i, var_i, 1, None,
                                op0=OP.arith_shift_right)
        nc.vector.tensor_scalar(rst_i, rst_i, -1, MAGIC, op0=OP.mult,
                                op1=OP.add)
        for _ in range(2):
            t = vtile("vnt")[:, c0:c1]
            nc.vector.tensor_tensor(t, rst, rst, op=OP.mult)
            nc.vector.tensor_tensor(t, t, var, op=OP.mult)
            nc.vector.tensor_scalar(t, t, -0.5, 1.5, op0=OP.mult,
                                    op1=OP.add)
            nc.vector.tensor_tensor(rst, rst, t, op=OP.mult)
        return mu, rst

    def ln_norm_simple(src_fn, mu, rst, write_out, c0=0, c1=None):
        c1 = SQ if c1 is None else c1
        for kt in range(KD):
            # alternate DVE/Pool so the (otherwise serial) normalize chain
            # runs on two engines; Pool is idle in the n2 tail
            eng = nc.vector if kt % 2 == 0 else nc.gpsimd
            t1 = tmpp.tile([128, SQ], F32, name="lnt", tag="lnt", bufs=4)
            eng.tensor_tensor(t1[:, c0:c1], src_fn(kt), mu,
                              op=OP.subtract)
            eng.tensor_tensor(t1[:, c0:c1], t1[:, c0:c1], rst,
                              op=OP.mult)
            write_out(kt, t1)

    # ---------------- LN chains ----------------
    def lna_r2(qc):
        """lna on (x + racc); then r2 = x + lna_out accumulated into xp."""
        def cast_lna(kt):
            rbf = sqp.tile([128, SQ], BF16, name="rbf", tag="rbf", bufs=2)
            nc.gpsimd.tensor_tensor(
                rbf[:], xp[kt][:, 1 + qc * SQ:1 + (qc + 1) * SQ],
                racc[kt][:, qs(qc)], op=OP.add)
            return rbf

        sp = ln_stats(cast_lna)
        mu, rst = ln_finish(sp)
        for kt in range(KD):
            xs = xp[kt][:, 1 + qc * SQ:1 + (qc + 1) * SQ]
            t1 = tmpp.tile([128, SQ], F32, name="lnt", tag="lnt", bufs=4)
            nc.vector.tensor_tensor(t1[:], xs, racc[kt][:, qs(qc)], op=OP.add)
            nc.vector.tensor_tensor(t1[:], t1[:], mu[:], op=OP.subtract)
            nc.vector.tensor_tensor(t1[:], t1[:], rst[:], op=OP.mult)
            t2 = tmpp.tile([128, SQ], F32, name="lnt", tag="lnt", bufs=4)
            nc.scalar.activation(t2[:], t1[:], AF.Identity,
                                 bias=ct["lnab"][:, kt:kt + 1],
                                 scale=ct["lnag"][:, kt:kt + 1])
            nc.gpsimd.tensor_tensor(xs, xs, t2[:], op=OP.add)

    h_holder = []

    def n1_chain(qc):
        h_bf = h_holder

        def cast_n1(kt):
            rbf = sqp.tile([128, SQ], BF16, name="rbf", tag="rbf", bufs=2)
            nc.gpsimd.tensor_copy(rbf[:],
                                  xp[kt][:, 1 + qc * SQ:1 + (qc + 1) * SQ])
            return rbf

        sp = ln_stats(cast_n1)
        mu, rst = ln_finish(sp)

        def w_n1(kt, t1):
            nc.scalar.activation(h_bf[kt][:, qs(qc)], t1[:], AF.Identity,
                                 bias=ct["n1b"][:, kt:kt + 1],
                                 scale=ct["n1g"][:, kt:kt + 1])

        ln_norm_simple(
            lambda kt: xp[kt][:, 1 + qc * SQ:1 + (qc + 1) * SQ], mu, rst, w_n1)

    # ========================================================================
    # schedule
    # ========================================================================
    for m in range(KD):
        qk_proj(m)
    _ao.extend(aop.tile([128, S], BF16, name=f"ao{m}", tag=f"ao{m}")
               for m in range(KD))
    h_holder.extend(xbf)    # reuse the dead x-bf16 tiles as h storage
    for hp in range(HP):
        attn(hp, 0)
    for m in range(KD):
        wo_step(m, 0)
    lna_r2(0)               # chains run on DVE/Pool under attn(qc1) below
    for hp in range(HP // 2):
        attn(hp, 1)
    n1_chain(0)
    for hp in range(HP // 2, HP):
        attn(hp, 1)
    expp.release()
    qkp.release()
    vtp.release()

    for m in range(KD):
        wo_step(m, 1)
    aop.release()
    lna_r2(1)               # chain overlaps FFN-W1(qc0)

    f1p = pool(name="ffn1", bufs=1, side="right")

    def ffn_w1(qc, mid=None):
        f1t = []
        for m in range(KF):
            if m == 8 and mid is not None:
                mid()
            wt = wd_tile()
            nc.sync.dma_start(wt[:], io["w1s"][:, m * D:(m + 1) * D])
            ps = ps_tile()
            for kt in range(KD):
                nc.tensor.matmul(ps[:], wt[:, kt * 128:(kt + 1) * 128],
                                 h_holder[kt][:, qs(qc)],
                                 start=(kt == 0), stop=(kt == KD - 1))
            t = f1p.tile([128, SQ], BF16, name=f"f1_{m}", tag=f"f1_{m}")
            nc.scalar.activation(t[:], ps[:], AF.Gelu,
                                 bias=ct["b1t"][:, m:m + 1])
            f1t.append(t)
        return f1t

    f1_qc0 = ffn_w1(0, mid=lambda: n1_chain(1))

    wf = pool(name="wf", bufs=1)
    fop = pool(name="fout", bufs=1)
    f2p = pool(name="ffn2", bufs=1, side="right")
    fout = [fop.tile([128, SQ], F32, name=f"fo{m}", tag=f"fo{m}")
            for m in range(KD)]

    def wf_tile():
        """Half of a W2/W3 contraction block: [128, F//2]."""
        return wf.tile([128, F // 2], BF16, name="wf", tag="wf", bufs=4)

    def big_mm(ps, wname, m, rhs_tiles):
        """Accumulate over KF k-tiles streaming weights in two half tiles."""
        for h in range(2):
            wt = wf_tile()
            nc.sync.dma_start(wt[:], io[wname][:, m * F + h * (F // 2):
                                               m * F + (h + 1) * (F // 2)])
            for j in range(KF // 2):
                kt = h * (KF // 2) + j
                nc.tensor.matmul(ps, wt[:, j * 128:(j + 1) * 128],
                                 rhs_tiles[kt][:],
                                 start=(kt == 0), stop=(kt == KF - 1))

    def ffn_rest(qc, f1t, mid=None):
        h_bf = h_holder
        f2t = []
        for m in range(KF):
            if m == 8 and mid is not None:
                mid()
            ps = ps_tile()
            big_mm(ps[:], "w2s", m, f1t)
            t = f2p.tile([128, SQ], BF16, name=f"f2_{m}", tag=f"f2_{m}")
            nc.scalar.activation(t[:], ps[:], AF.Gelu,
                                 bias=ct["b2t"][:, m:m + 1])
            f2t.append(t)
        # W3 + gate, with incremental n2 statistics per output tile
        n2sp = pp_tile()
        for m in range(KD):
            wtg = wd_tile()
            nc.sync.dma_start(wtg[:], io["wgs"][:, m * D:(m + 1) * D])
            gp = pp_tile()
            psg, ps3 = gp[:, 0, :], gp[:, 1, :]
            for kt in range(KD):
                nc.tensor.matmul(psg, wtg[:, kt * 128:(kt + 1) * 128],
                                 h_bf[kt][:, qs(qc)],
                                 start=(kt == 0), stop=(kt == KD - 1))
            gat = tmpp.tile([128, SQ], BF16, name="gat", tag="gat", bufs=2)
            nc.scalar.activation(gat[:], psg, AF.Sigmoid,
                                 bias=ct["bgt"][:, m:m + 1])
            big_mm(ps3, "w3s", m, f2t)
            t = tmpp.tile([128, SQ], F32, name="f3t", tag="f3t", bufs=2)
            nc.vector.scalar_tensor_tensor(t[:], ps3, ct["b3t"][:, m:m + 1],
                                           gat[:], op0=OP.add, op1=OP.mult)
            nc.vector.tensor_tensor(fout[m][:], t[:],
                                    h_bf[m][:, qs(qc)], op=OP.add)
            # incremental n2 stats for this feature tile
            rbf = sqp.tile([128, SQ], BF16, name="rbf", tag="rbf", bufs=2)
            nc.gpsimd.tensor_copy(rbf[:], fout[m][:])
            sq_t = sqp.tile([128, SQ], BF16, name="sq", tag="sq", bufs=2)
            nc.scalar.square(sq_t[:], rbf[:])
            st_, sp_ = (m == 0), (m == KD - 1)
            nc.tensor.matmul(n2sp[:, 0, :], ones_bf[:, 0:128], rbf[:],
                             start=st_, stop=sp_)
            nc.tensor.matmul(n2sp[:, 1, :], ones_bf[:, 0:128], sq_t[:],
                             start=st_, stop=sp_)
        return n2sp

    def emit_n2(qc, n2sp):
        mu, rst = ln_finish(n2sp)

        def w_n2(kt, t1):
            stg = stgp.tile([128, SQ], F16, name="stg", tag="stg", bufs=2)
            nc.scalar.activation(stg[:], t1[:], AF.Identity,
                                 bias=ct["n2b"][:, kt:kt + 1],
                                 scale=ct["n2g"][:, kt:kt + 1])
            nc.sync.dma_start(io["outT"][kt * 128:(kt + 1) * 128, qs(qc)],
                              stg[:])

        ln_norm_simple(lambda kt: fout[kt][:], mu, rst, w_n2)

    n2sp0 = ffn_rest(0, f1_qc0)
    emit_n2(0, n2sp0)
    f1_qc1 = ffn_w1(1)
    n2sp1 = ffn_rest(1, f1_qc1)
    emit_n2(1, n2sp1)

    f2p.release()
    f1p.release()
    fop.release()
    wf.release()
    xbfp.release()
    xpp.release()
    rp.release()
    wd.release()
    stgp.release()
    smallp.release()
    sqp.release()
    vecp.release()
    tmpp.release()
    consts.release()
    psum.release()


# ------------------------------------------------------------------
# host side
# ------------------------------------------------------------------

def _shuffle_w(w):
    """[K, E] -> [128, (E//128)*K] bf16 so that slice [:, m*K:(m+1)*K]
    viewed as [128, K//128, 128] gives lhsT tiles w[kt*128+p, m*128+c]."""
    K, E = w.shape
    r = np.asarray(w).reshape(K // 128, 128, E // 128, 128).transpose(1, 2, 0, 3)
    return np.ascontiguousarray(r.reshape(128, (E // 128) * K)).astype(
        ml_dtypes.bfloat16)


def _ptable(b):
    """[E] -> [128, E//128] per-partition scalar table."""
    return np.ascontiguousarray(np.asarray(b, np.float32).reshape(-1, 128).T)


def _declare_io(nc, cfg, consts=None):
    """xT/outT are per-call I/O; every weight tensor is embedded in the
    NEFF as a Const DRAM tensor (DMA'd to HBM once at model-load time) so
    repeated executions only ship the activation tensors."""
    S, D, F, KD, KF = cfg.S, cfg.D, cfg.F, cfg.KD, cfg.KF
    io = {}

    def inp(name, shape, dt):
        if consts is not None and name in consts:
            arr = np.ascontiguousarray(consts[name])
            assert list(arr.shape) == list(shape), (name, arr.shape, shape)
            io[name] = nc.inline_tensor(arr, name=name).ap()
        else:
            io[name] = nc.dram_tensor(name, shape, dt, kind="ExternalInput").ap()

    inp("xT", [D, S], F16)
    inp("wqs", [128, KD * D], BF16)
    inp("wks", [128, KD * D], BF16)
    inp("wv", [D, D], BF16)
    inp("wos", [128, KD * D], BF16)
    inp("w1s", [128, KF * D], BF16)
    inp("w2s", [128, KF * F], BF16)
    inp("wgs", [128, KD * D], BF16)
    inp("w3s", [128, KD * F], BF16)
    inp("cpk", [128, 16 * KD + 2 * KF], F32)
    inp("bvr", [1, D], BF16)
    io["outT"] = nc.dram_tensor("outT", [D, S], F16, kind="ExternalOutput").ap()
    return io


def build_shared_inputs(inputs, cfg):
    """Everything except xT (identical across cores)."""
    f32 = np.float32
    g = {k: np.asarray(v) for k, v in inputs.items()}
    ptabs = [
        _ptable(g["bq"]), _ptable(g["bk"]),
        _ptable(np.asarray(g["bo"], f32) + 0.3 * np.asarray(g["conv_b"], f32)),
        _ptable(0.3 * np.asarray(g["conv_w"], f32)[:, 0]),
        _ptable(0.3 * np.asarray(g["conv_w"], f32)[:, 1]),
        _ptable(0.3 * np.asarray(g["conv_w"], f32)[:, 2]),
        _ptable(g["b3"]), _ptable(g["bg"]),
        _ptable(g["lna_g"]), _ptable(g["lna_b"]),
        _ptable(g["n1_g"]), _ptable(g["n1_b"]),
        _ptable(g["n2_g"]), _ptable(g["n2_b"]),
        _ptable(np.asarray(g["lna_b"], f32)
                / np.where(np.abs(np.asarray(g["lna_g"], f32)) < 1e-20,
                           1.0, np.asarray(g["lna_g"], f32))),
        _ptable(np.asarray(g["n1_b"], f32)
                / np.where(np.abs(np.asarray(g["n1_g"], f32)) < 1e-20,
                           1.0, np.asarray(g["n1_g"], f32))),
        _ptable(g["b1"]), _ptable(g["b2"]),
    ]
    sh = {
        "wqs": _shuffle_w(g["Wq"]), "wks": _shuffle_w(g["Wk"]),
        "wv": np.ascontiguousarray(g["Wv"]).astype(ml_dtypes.bfloat16),
        "wos": _shuffle_w(g["Wo"]), "w1s": _shuffle_w(g["W1"]),
        "w2s": _shuffle_w(g["W2"]), "w3s": _shuffle_w(g["W3"]),
        "wgs": _shuffle_w(g["Wg"]),
        "cpk": np.ascontiguousarray(np.concatenate(ptabs, axis=1)),
        "bvr": np.ascontiguousarray(
            np.asarray(g["bv"], f32).reshape(1, cfg.D)).astype(
                ml_dtypes.bfloat16),
    }
    return sh


_CACHE = {}


def _weights_fingerprint(inputs):
    """Content hash of every non-x input. The NEFF embeds the weights, so
    a changed weight set must rebuild (and recompile) the kernel."""
    import hashlib
    h = hashlib.blake2b(digest_size=16)
    for k in sorted(inputs):
        if k == "x":
            continue
        a = np.ascontiguousarray(inputs[k])
        h.update(k.encode())
        h.update(str(a.shape).encode())
        h.update(str(a.dtype).encode())
        h.update(a.tobytes())
    return h.hexdigest()


def _get_nc(inputs):
    key = _weights_fingerprint(inputs)
    ent = _CACHE.get("nc")
    if ent is None or ent[0] != key:
        shared = build_shared_inputs(inputs, FULL)
        nc = bass.Bass("TRN2", target_bir_lowering=False, debug=False)
        io = _declare_io(nc, FULL, consts=shared)
        with _TC(nc) as tc:
            emit(tc, FULL, io)
        _CACHE["nc"] = (key, nc)
    return _CACHE["nc"][1]


def _get_exec(inputs):
    """Persistent jitted executor: one bass_exec custom call shard_mapped
    over the 8 cores.  Rebuilt only when the weight set changes."""
    import jax
    from jax.sharding import Mesh, PartitionSpec
    from jax.experimental.shard_map import shard_map
    from concourse import bass2jax

    key = _weights_fingerprint(inputs)
    ent = _CACHE.get("exec")
    if ent is not None and ent[0] == key:
        return ent[1]

    bass2jax.install_neuronx_cc_hook()
    nc = _get_nc(inputs)
    pid = nc.partition_id_tensor.name if nc.partition_id_tensor else None
    in_names, out_names, out_avals = [], [], []
    for alloc in nc.m.functions[0].allocations:
        if not isinstance(alloc, mybir.MemoryLocationSet):
            continue
        name = alloc.memorylocations[0].name
        if alloc.kind == "ExternalInput":
            if name != pid:
                in_names.append(name)
        elif alloc.kind == "ExternalOutput":
            out_names.append(name)
            out_avals.append(jax.core.ShapedArray(
                tuple(alloc.tensor_shape), mybir.dt.np(alloc.dtype)))
    all_names = in_names + ([pid] if pid is not None else [])
    assert in_names == ["xT"] and out_names == ["outT"]

    def _body(*args):
        operands = list(args)
        if pid is not None:
            operands.append(bass2jax.partition_id_tensor())
        return tuple(bass2jax._bass_exec_p.bind(
            *operands, out_avals=tuple(out_avals), in_names=tuple(all_names),
            out_names=tuple(out_names), lowering_input_output_aliases=(),
            sim_require_finite=True, sim_require_nnan=True, nc=nc))

    mesh = Mesh(np.asarray(jax.devices()[:N_CORES]), ("core",))
    sharded = jax.jit(
        shard_map(_body, mesh=mesh, in_specs=(PartitionSpec("core"),),
                  out_specs=(PartitionSpec("core"),), check_rep=False),
        keep_unused=True)
    _CACHE["exec"] = (key, sharded)
    return sharded


def kernel(**inputs):
    import jax

    cfg = FULL
    sharded = _get_exec(inputs)
    x = np.asarray(inputs["x"], dtype=np.float32)
    B = x.shape[0]
    assert B == N_CORES
    xc = np.concatenate([x[b].T.astype(np.float16, order="C")
                         for b in range(B)], axis=0)
    out = sharded(jax.device_put(xc))
    o0 = np.asarray(out[0]).astype(np.float32).reshape(B, cfg.D, cfg.S)
    return np.ascontiguousarray(o0.transpose(0, 2, 1))



# revision 21
# speedup vs baseline: 1.0972x; 1.0593x over previous
"""Trainium2 Bass kernel for an enhanced transformer layer.

Strategy: data-parallel over batch (B=8 -> one batch element per NeuronCore,
no collectives).  Activations are kept feature-major ([D, S] with the
contraction dim on partitions) so every linear layer consumes weights in
natural [K, E] layout as the stationary operand.  Matmuls run in bf16 with
fp32 PSUM accumulation.

I/O strategy: all weight tensors are embedded in the NEFF as Const DRAM
tensors (DMA'd to HBM once at model-load time), so per-execution bindings
are just xT (fp16 in) and outT (fp16 out) -- 2 MB each per core.  The NEFF
is rebuilt (keyed on a content hash of the weights) if kernel() is called
with a different weight set.  fp16 transport adds ~2^-11 relative
quantization noise on x and out, negligible against the bf16 matmul noise.

Schedule: the layer is software-pipelined over the two S/2 column chunks
(qc) so LayerNorm / softmax vector work overlaps matmul streams:
  v-proj (staged x arrival) -> qk-proj -> attn(qc0) -> [attn(hp, qc1) |
  Wo(qc0)] -> lna(qc0) | Wo(qc1).a -> n1(qc0) | Wo(qc1).b ->
  FFN-W1(qc0) | lna(qc1) -> FFN-W2(qc0) | n1(qc1) -> FFN-W3/gate(qc0) with
  incremental n2(qc0) stats -> n2(qc0) | FFN(qc1) -> n2(qc1).
LayerNorm statistics use M=128 ones-matmuls that produce row-broadcast sums
directly (no scalar-row chains, no separate broadcast matmuls), and
1/sqrt(var) is computed on DVE via Newton iteration so the ACT engine never
loads the sqrt table (a table-set switch costs ~2.7us).  Softmax
denominators are likewise accumulated as broadcast ones-matmuls; exp is
evaluated on paired PSUM banks ([128, 2*SQ] per instruction) to halve ACT
dispatch count in the attention phase.  The depthwise-conv residual and
h=x+attn adds run on GPSIMD (Pool).  The attention residual accumulator
(conv + Wo output, without x) is kept in bf16; x is added back in fp32 at
LN time.  All small constants ship in one packed DMA (dma_start dispatch
costs ~0.65us each on the sync engine).
"""

import math

import numpy as np
import ml_dtypes

import concourse.bass as bass
import concourse.tile as tile
from concourse import mybir
from concourse.alu_op_type import AluOpType
from bass_rust import ScopedClock

F32 = mybir.dt.float32
I32 = mybir.dt.int32
BF16 = mybir.dt.bfloat16
F16 = mybir.dt.float16
AF = mybir.ActivationFunctionType
OP = AluOpType

EPS = 1e-5
N_CORES = 8
MAGIC = 0x5F3759DF


class CFG:
    def __init__(self, S=1024, D=1024, F=4096, H=16):
        self.S, self.D, self.F, self.H = S, D, F, H
        self.DK = D // H              # head dim (must be 64)
        self.KD = D // 128            # feature tiles of model dim
        self.KF = F // 128            # feature tiles of ffn dim
        self.SQ = min(512, S)         # moving-dim chunk
        self.NQ = S // self.SQ
        self.NKT = S // 128           # key/sequence tiles
        self.VC = min(512, D)         # v-projection output chunk
        self.NVC = D // self.VC
        assert self.DK == 64 and H % 2 == 0


FULL = CFG()


def _split_excess_waits(nc, max_waits=1):
    """Walrus in this container rejects >2 sync waits per instruction.
    Hoist excess waits onto same-engine nops inserted just before."""
    cnt = 0
    for fn in nc.m.functions:
        for bb in fn.blocks:
            insts = list(bb.instructions)
            out = []
            for inst in insts:
                si = inst.sync_info
                waits = list(si.on_wait) if si and si.on_wait else []
                if len(waits) > max_waits:
                    extra = waits[:-max_waits]
                    si.on_wait = waits[-max_waits:]
                    for i in range(0, len(extra), max_waits):
                        cnt += 1
                        out.append(mybir.InstNoOp(
                            name=f"waitsplit{cnt}_{inst.name}",
                            engine=inst.engine, ins=[], outs=[],
                            sync_info=mybir.SyncInfo(
                                on_wait=extra[i:i + max_waits], on_update=[]),
                        ))
                out.append(inst)
            if cnt:
                bb.instructions = out
    return cnt


class _TC(tile.TileContext):
    """TileContext whose exit drain spreads semaphore waits over several
    sync-engine nops -- this container's walrus rejects >2 sync waits on a
    single CTRL instruction."""

    def __exit__(self, *a):
        r = super().__exit__(*a)
        _split_excess_waits(self.nc)
        return r

    def _drain_and_barrier(self, tick_clock, wait_clock):
        nc = self.nc
        drain_inst = nc.sync.drain()
        wait_clock.add_sem_waits(
            drain_inst.ins, ScopedClock({None: tick_clock.global_clock})
        )
        si = drain_inst.ins.sync_info
        waits = list(si.on_wait) if si and si.on_wait else []
        if len(waits) > 1:
            si.on_wait = waits[:1]
            for w in waits[1:]:
                nop = nc.sync.nop(nofuse=True)
                nsi = nop.ins.sync_info
                if nsi is None:
                    nop.ins.sync_info = mybir.SyncInfo(on_wait=[w], on_update=[])
                else:
                    nsi.on_wait = [w]
        nc.all_engine_barrier()
        popped = nc._tile_sem_poison_stack.pop()
        assert popped is self._sem_poison
        nc.clear_and_free_semaphores(list(self.sems.allocated().values()))
        nc.all_engine_barrier()


def emit(tc, cfg, io):
    nc = tc.nc
    S, D, F, H = cfg.S, cfg.D, cfg.F, cfg.H
    DK, KD, KF = cfg.DK, cfg.KD, cfg.KF
    SQ, NQ, NKT = cfg.SQ, cfg.NQ, cfg.NKT
    VC, NVC = cfg.VC, cfg.NVC
    HP = H // 2

    def qs(qc):
        return slice(qc * SQ, (qc + 1) * SQ)

    pool = tc.alloc_tile_pool

    # ======== pools: alloc order is EXACT reverse death order per side ====
    consts = pool(name="consts", bufs=1)
    tmpp = pool(name="tmp", bufs=1)
    vecp = pool(name="vec", bufs=1)
    sqp = pool(name="sq", bufs=1)
    smallp = pool(name="small", bufs=1)
    stgp = pool(name="stg", bufs=1)
    wd = pool(name="wd", bufs=1)
    rp = pool(name="racc", bufs=1)           # bf16 conv+Wo residual
    xpp = pool(name="xp", bufs=1)
    xbfp = pool(name="xbf", bufs=1)          # x bf16, then reused as h
    vtp = pool(name="vt", bufs=1, side="right")
    wvp = pool(name="wv", bufs=1, side="right")
    xhp = pool(name="xh", bufs=1, side="right")  # fp16 x staging
    aop = pool(name="aop", bufs=1)
    qkp = pool(name="qk", bufs=1)
    expp = pool(name="expT", bufs=1)
    psum = pool(name="ps", bufs=1, space="PSUM")

    # ---------------- constants (one packed DMA) ----------------
    cpk = consts.tile([128, 16 * KD + 2 * KF], F32, name="cpk", tag="cpk")
    ct = {}
    off = 0
    for cname in ("bq", "bk", "bocb", "cw0", "cw1", "cw2", "b3t", "bgt",
                  "lnag", "lnab", "n1g", "n1b", "n2g", "n2b", "bdla", "bdn1"):
        ct[cname] = cpk[:, off:off + KD]
        off += KD
    for cname in ("b1t", "b2t"):
        ct[cname] = cpk[:, off:off + KF]
        off += KF
    ones_bf = consts.tile([128, 128], BF16, name="onesbf", tag="onesbf")
    nc.vector.memset(ones_bf[:], 1.0)
    bvr_bf = consts.tile([1, D], BF16, name="bvrbf", tag="bvrbf")
    nc.scalar.dma_start(bvr_bf[:], io["bvr"][:, :])
    nc.scalar.dma_start(cpk[:], io["cpk"][:, :])

    def ps_tile():
        return psum.tile([128, SQ], F32, name="ps", tag="ps", bufs=2)

    def pp_tile():
        return psum.tile([128, 2, SQ], F32, name="pp", tag="pp", bufs=3)

    # ---------------- x + wv loads, staged for earliest v-proj ------------
    xp, xbf = [], []
    for kt in range(KD):
        t = xpp.tile([128, S + 2], F32, name=f"xp{kt}", tag=f"xp{kt}")
        xp.append(t)
        b = xbfp.tile([128, S], BF16, name=f"xbf{kt}", tag=f"xbf{kt}")
        xbf.append(b)
    wv = [wvp.tile([128, D], BF16, name=f"wv{kt}", tag=f"wv{kt}")
          for kt in range(KD)]
    # wv loads dispatch on the (startup-idle) ACT engine's HW DMA queue so
    # they don't serialize behind the x staging on the sync engine.
    for kt in range(KD):
        nc.scalar.dma_start(wv[kt][:], io["wv"][kt * 128:(kt + 1) * 128, :])
    # x arrives fp16; stage through a rotating SBUF tile, then fan out to
    # the bf16 matmul copy (DVE) and the fp32 conv/LN copy (DVE).
    def stage_x(kt, c0, c1):
        xh = xhp.tile([128, 512], F16, name="xh", tag="xh",
                      bufs=3)[:, :c1 - c0]
        nc.sync.dma_start(xh, io["xT"][kt * 128:(kt + 1) * 128, c0:c1])
        nc.vector.tensor_copy(xbf[kt][:, c0:c1], xh)
        nc.vector.tensor_copy(xp[kt][:, 1 + c0:1 + c1], xh)

    # first seq tile of x (enables v-proj st=0), then the rest
    for kt in range(KD):
        stage_x(kt, 0, 128)
    for kt in range(KD):
        stage_x(kt, 128, SQ)
    for kt in range(KD):
        stage_x(kt, SQ, S)
        nc.vector.memset(xp[kt][:, 0:1], 0.0)
        nc.vector.memset(xp[kt][:, S + 1:S + 2], 0.0)

    # ---------------- v projection (x stationary, wv moving) --------------
    v_t = [vtp.tile([128, D], BF16, name=f"vt{st}", tag=f"vt{st}")
           for st in range(NKT)]
    for st in range(NKT):
        for ec in range(NVC):
            ps = ps_tile()
            for kt in range(KD):
                nc.tensor.matmul(ps[:, 0:VC],
                                 xbf[kt][:, st * 128:(st + 1) * 128],
                                 wv[kt][:, ec * VC:(ec + 1) * VC],
                                 start=(kt == 0), stop=False)
            # bv bias via K=1 ones-row matmul into the same accumulation
            nc.tensor.matmul(ps[:, 0:VC], ones_bf[0:1, 0:128],
                             bvr_bf[0:1, ec * VC:(ec + 1) * VC],
                             start=False, stop=True)
            nc.vector.tensor_copy(v_t[st][:, ec * VC:(ec + 1) * VC],
                                  ps[:, 0:VC])
    xhp.release()
    wvp.release()

    # ---------------- conv residual on GPSIMD: racc = 0.3*conv ------------
    # (x itself is NOT accumulated here; it is added back in fp32 at LN
    #  time.  racc is bf16: it only ever holds conv + attention output.)
    racc = [rp.tile([128, S], BF16, name=f"ra{kt}", tag=f"ra{kt}")
            for kt in range(KD)]
    for kt in range(KD):
        for qc in range(NQ):
            o = qc * SQ
            xl, xc, xr = (xp[kt][:, o:o + SQ], xp[kt][:, o + 1:o + SQ + 1],
                          xp[kt][:, o + 2:o + SQ + 2])
            t1 = tmpp.tile([128, SQ], F32, name="lnt", tag="lnt", bufs=4)
            nc.vector.tensor_scalar(t1[:], xl, ct["cw0"][:, kt:kt + 1], None,
                                    op0=OP.mult)
            t2 = tmpp.tile([128, SQ], F32, name="lnt", tag="lnt", bufs=4)
            nc.vector.scalar_tensor_tensor(t2[:], xc, ct["cw1"][:, kt:kt + 1],
                                           t1[:], op0=OP.mult, op1=OP.add)
            nc.vector.scalar_tensor_tensor(racc[kt][:, qs(qc)], xr,
                                           ct["cw2"][:, kt:kt + 1], t2[:],
                                           op0=OP.mult, op1=OP.add)

    # ---------------- q/k projection (paired psum: both qc at once) -------
    q_fm = [qkp.tile([128, S], BF16, name=f"q{m}", tag=f"q{m}")
            for m in range(KD)]
    k_fm = [qkp.tile([128, S], BF16, name=f"k{m}", tag=f"k{m}")
            for m in range(KD)]

    def wd_tile():
        return wd.tile([128, D], BF16, name="wd", tag="wd", bufs=3)

    def qk_proj(m):
        for wname, bias, dst in (("wqs", ct["bq"], q_fm),
                                 ("wks", ct["bk"], k_fm)):
            wt = wd_tile()
            nc.sync.dma_start(wt[:], io[wname][:, m * D:(m + 1) * D])
            pp = pp_tile()
            for qc in range(NQ):
                for kt in range(KD):
                    nc.tensor.matmul(pp[:, qc, :],
                                     wt[:, kt * 128:(kt + 1) * 128],
                                     xbf[kt][:, qs(qc)],
                                     start=(kt == 0), stop=(kt == KD - 1))
            nc.scalar.activation(dst[m][:], pp[:], AF.Identity,
                                 bias=bias[:, m:m + 1])

    # ---------------- attention ----------------
    inv_sqrt_dk = 1.0 / math.sqrt(DK)
    _ao = []

    def attn(hp, qc):
        attnout = _ao
        h0, h1 = 2 * hp, 2 * hp + 1
        eps_ = []
        for kt in range(NKT):
            pp = pp_tile()
            nc.tensor.matmul(pp[:, 0, :],
                             k_fm[hp][0:64, kt * 128:(kt + 1) * 128],
                             q_fm[hp][0:64, qs(qc)], start=True, stop=True)
            nc.tensor.matmul(pp[:, 1, :],
                             k_fm[hp][64:128, kt * 128:(kt + 1) * 128],
                             q_fm[hp][64:128, qs(qc)], start=True, stop=True)
            ep = expp.tile([128, 2, SQ], BF16, name="exp", tag="exp", bufs=9)
            nc.scalar.activation(ep[:], pp[:], AF.Exp, scale=inv_sqrt_dk)
            eps_.append(ep)
        den_t, U_t = ps_tile(), ps_tile()
        den, U = den_t[:, :], U_t[:, :]
        for kt in range(NKT):
            nc.tensor.matmul(den[0:64, :], ones_bf[:, 0:64], eps_[kt][:, 0, :],
                             start=(kt == 0), stop=(kt == NKT - 1))
        for kt in range(NKT):
            nc.tensor.matmul(den[64:128, :], ones_bf[:, 0:64],
                             eps_[kt][:, 1, :],
                             start=(kt == 0), stop=(kt == NKT - 1))
        for kt in range(NKT):
            nc.tensor.matmul(U[0:64, :], v_t[kt][:, h0 * DK:(h0 + 1) * DK],
                             eps_[kt][:, 0, :],
                             start=(kt == 0), stop=(kt == NKT - 1))
        for kt in range(NKT):
            nc.tensor.matmul(U[64:128, :], v_t[kt][:, h1 * DK:(h1 + 1) * DK],
                             eps_[kt][:, 1, :],
                             start=(kt == 0), stop=(kt == NKT - 1))
        recb = smallp.tile([128, SQ], F32, name="recb", tag="recb", bufs=2)
        nc.vector.reciprocal(recb[:], den)
        nc.vector.tensor_tensor(attnout[hp][:, qs(qc)], U, recb[:],
                                op=OP.mult)

    # ---------------- Wo accumulate into racc ----------------
    def wo_step(m, qc):
        attnout = _ao
        wt = wd_tile()
        nc.sync.dma_start(wt[:], io["wos"][:, m * D:(m + 1) * D])
        ps = ps_tile()
        for kt in range(KD):
            nc.tensor.matmul(ps[:], wt[:, kt * 128:(kt + 1) * 128],
                             attnout[kt][:, qs(qc)],
                             start=(kt == 0), stop=(kt == KD - 1))
        nc.vector.scalar_tensor_tensor(racc[m][:, qs(qc)], ps[:],
                                       ct["bocb"][:, m:m + 1],
                                       racc[m][:, qs(qc)],
                                       op0=OP.add, op1=OP.add)

    # ---------------- layernorm helpers (broadcast stats) ----------------
    def vtile(tag, dt=F32):
        return vecp.tile([128, SQ], dt, name=tag, tag=tag, bufs=1)

    def ln_stats(cast_fn):
        """Returns psum pair (ssum at [:,0,:], ssq at [:,1,:])."""
        sp = pp_tile()
        for kt in range(KD):
            rbf = cast_fn(kt)
            sq_t = sqp.tile([128, SQ], BF16, name="sq", tag="sq", bufs=2)
            nc.vector.tensor_tensor(sq_t[:], rbf[:], rbf[:], op=OP.mult)
            st_, sp_ = (kt == 0), (kt == KD - 1)
            nc.tensor.matmul(sp[:, 0, :], ones_bf[:, 0:128], rbf[:],
                             start=st_, stop=sp_)
            nc.tensor.matmul(sp[:, 1, :], ones_bf[:, 0:128], sq_t[:],
                             start=st_, stop=sp_)
        return sp

    def ln_finish(sp, c0=0, c1=None):
        """mu, rstd [128, c1-c0] from the stats pair (Newton rsqrt on DVE)."""
        c1 = SQ if c1 is None else c1
        mu_t, var_t, rst_t = vtile("vmu"), vtile("vvar"), vtile("vrst")
        mu, var, rst = mu_t[:, c0:c1], var_t[:, c0:c1], rst_t[:, c0:c1]
        nc.vector.tensor_scalar_mul(mu, sp[:, 0, c0:c1], 1.0 / D)
        nc.vector.tensor_tensor(var, mu, mu, op=OP.mult)
        nc.vector.scalar_tensor_tensor(var, sp[:, 1, c0:c1], 1.0 / D, var,
                                       op0=OP.mult, op1=OP.subtract)
        nc.vector.tensor_scalar_add(var, var, EPS)
        rst_i = rst_t[:].bitcast(I32)[:, c0:c1]
        var_i = var_t[:].bitcast(I32)[:, c0:c1]
        nc.vector.tensor_scalar(rst_i, var_i, 1, None,
                                op0=OP.arith_shift_right)
        nc.vector.tensor_scalar(rst_i, rst_i, -1, MAGIC, op0=OP.mult,
                                op1=OP.add)
        for _ in range(2):
            t = vtile("vnt")[:, c0:c1]
            nc.vector.tensor_tensor(t, rst, rst, op=OP.mult)
            nc.vector.tensor_tensor(t, t, var, op=OP.mult)
            nc.vector.tensor_scalar(t, t, -0.5, 1.5, op0=OP.mult,
                                    op1=OP.add)
            nc.vector.tensor_tensor(rst, rst, t, op=OP.mult)
        return mu, rst

    def ln_norm_simple(src_fn, mu, rst, write_out, c0=0, c1=None):
        c1 = SQ if c1 is None else c1
        for kt in range(KD):
            # alternate DVE/Pool so the (otherwise serial) normalize chain
            # runs on two engines; Pool is idle in the n2 tail
            eng = nc.vector if kt % 2 == 0 else nc.gpsimd
            t1 = tmpp.tile([128, SQ], F32, name="lnt", tag="lnt", bufs=4)
            eng.tensor_tensor(t1[:, c0:c1], src_fn(kt), mu,
                              op=OP.subtract)
            eng.tensor_tensor(t1[:, c0:c1], t1[:, c0:c1], rst,
                              op=OP.mult)
            write_out(kt, t1)

    # ---------------- LN chains ----------------
    def lna_r2(qc):
        """lna on (x + racc); then r2 = x + lna_out accumulated into xp."""
        def cast_lna(kt):
            rbf = sqp.tile([128, SQ], BF16, name="rbf", tag="rbf", bufs=2)
            nc.gpsimd.tensor_tensor(
                rbf[:], xp[kt][:, 1 + qc * SQ:1 + (qc + 1) * SQ],
                racc[kt][:, qs(qc)], op=OP.add)
            return rbf

        sp = ln_stats(cast_lna)
        mu, rst = ln_finish(sp)
        for kt in range(KD):
            xs = xp[kt][:, 1 + qc * SQ:1 + (qc + 1) * SQ]
            t1 = tmpp.tile([128, SQ], F32, name="lnt", tag="lnt", bufs=4)
            nc.vector.tensor_tensor(t1[:], xs, racc[kt][:, qs(qc)], op=OP.add)
            nc.vector.tensor_tensor(t1[:], t1[:], mu[:], op=OP.subtract)
            nc.vector.tensor_tensor(t1[:], t1[:], rst[:], op=OP.mult)
            t2 = tmpp.tile([128, SQ], F32, name="lnt", tag="lnt", bufs=4)
            nc.scalar.activation(t2[:], t1[:], AF.Identity,
                                 bias=ct["lnab"][:, kt:kt + 1],
                                 scale=ct["lnag"][:, kt:kt + 1])
            nc.gpsimd.tensor_tensor(xs, xs, t2[:], op=OP.add)

    h_holder = []

    def n1_chain(qc):
        h_bf = h_holder

        def cast_n1(kt):
            rbf = sqp.tile([128, SQ], BF16, name="rbf", tag="rbf", bufs=2)
            nc.gpsimd.tensor_copy(rbf[:],
                                  xp[kt][:, 1 + qc * SQ:1 + (qc + 1) * SQ])
            return rbf

        sp = ln_stats(cast_n1)
        mu, rst = ln_finish(sp)

        def w_n1(kt, t1):
            nc.scalar.activation(h_bf[kt][:, qs(qc)], t1[:], AF.Identity,
                                 bias=ct["n1b"][:, kt:kt + 1],
                                 scale=ct["n1g"][:, kt:kt + 1])

        ln_norm_simple(
            lambda kt: xp[kt][:, 1 + qc * SQ:1 + (qc + 1) * SQ], mu, rst, w_n1)

    # ========================================================================
    # schedule
    # ========================================================================
    for m in range(KD):
        qk_proj(m)
    _ao.extend(aop.tile([128, S], BF16, name=f"ao{m}", tag=f"ao{m}")
               for m in range(KD))
    h_holder.extend(xbf)    # reuse the dead x-bf16 tiles as h storage
    for hp in range(HP):
        attn(hp, 0)
    for m in range(KD):
        wo_step(m, 0)
    lna_r2(0)               # chains run on DVE/Pool under attn(qc1) below
    for hp in range(HP // 2):
        attn(hp, 1)
    n1_chain(0)
    for hp in range(HP // 2, HP):
        attn(hp, 1)
    expp.release()
    qkp.release()
    vtp.release()

    for m in range(KD):
        wo_step(m, 1)
    aop.release()
    lna_r2(1)               # chain overlaps FFN-W1(qc0)

    f1p = pool(name="ffn1", bufs=1, side="right")

    def ffn_w1(qc, mid=None):
        f1t = []
        for m in range(KF):
            if m == 8 and mid is not None:
                mid()
            wt = wd_tile()
            nc.sync.dma_start(wt[:], io["w1s"][:, m * D:(m + 1) * D])
            ps = ps_tile()
            for kt in range(KD):
                nc.tensor.matmul(ps[:], wt[:, kt * 128:(kt + 1) * 128],
                                 h_holder[kt][:, qs(qc)],
                                 start=(kt == 0), stop=(kt == KD - 1))
            t = f1p.tile([128, SQ], BF16, name=f"f1_{m}", tag=f"f1_{m}")
            nc.scalar.activation(t[:], ps[:], AF.Gelu,
                                 bias=ct["b1t"][:, m:m + 1])
            f1t.append(t)
        return f1t

    f1_qc0 = ffn_w1(0, mid=lambda: n1_chain(1))

    wf = pool(name="wf", bufs=1)
    fop = pool(name="fout", bufs=1)
    f2p = pool(name="ffn2", bufs=1, side="right")
    fout = [fop.tile([128, SQ], F32, name=f"fo{m}", tag=f"fo{m}")
            for m in range(KD)]

    def wf_tile():
        """Half of a W2/W3 contraction block: [128, F//2]."""
        return wf.tile([128, F // 2], BF16, name="wf", tag="wf", bufs=4)

    def big_mm(ps, wname, m, rhs_tiles):
        """Accumulate over KF k-tiles streaming weights in two half tiles."""
        for h in range(2):
            wt = wf_tile()
            nc.sync.dma_start(wt[:], io[wname][:, m * F + h * (F // 2):
                                               m * F + (h + 1) * (F // 2)])
            for j in range(KF // 2):
                kt = h * (KF // 2) + j
                nc.tensor.matmul(ps, wt[:, j * 128:(j + 1) * 128],
                                 rhs_tiles[kt][:],
                                 start=(kt == 0), stop=(kt == KF - 1))

    def ffn_rest(qc, f1t, mid=None):
        h_bf = h_holder
        f2t = []
        for m in range(KF):
            if m == 8 and mid is not None:
                mid()
            ps = ps_tile()
            big_mm(ps[:], "w2s", m, f1t)
            t = f2p.tile([128, SQ], BF16, name=f"f2_{m}", tag=f"f2_{m}")
            nc.scalar.activation(t[:], ps[:], AF.Gelu,
                                 bias=ct["b2t"][:, m:m + 1])
            f2t.append(t)
        # W3 + gate, with incremental n2 statistics per output tile
        n2sp = pp_tile()
        for m in range(KD):
            wtg = wd_tile()
            nc.sync.dma_start(wtg[:], io["wgs"][:, m * D:(m + 1) * D])
            gp = pp_tile()
            psg, ps3 = gp[:, 0, :], gp[:, 1, :]
            for kt in range(KD):
                nc.tensor.matmul(psg, wtg[:, kt * 128:(kt + 1) * 128],
                                 h_bf[kt][:, qs(qc)],
                                 start=(kt == 0), stop=(kt == KD - 1))
            gat = tmpp.tile([128, SQ], BF16, name="gat", tag="gat", bufs=2)
            nc.scalar.activation(gat[:], psg, AF.Sigmoid,
                                 bias=ct["bgt"][:, m:m + 1])
            big_mm(ps3, "w3s", m, f2t)
            t = tmpp.tile([128, SQ], F32, name="f3t", tag="f3t", bufs=2)
            nc.vector.scalar_tensor_tensor(t[:], ps3, ct["b3t"][:, m:m + 1],
                                           gat[:], op0=OP.add, op1=OP.mult)
            nc.vector.tensor_tensor(fout[m][:], t[:],
                                    h_bf[m][:, qs(qc)], op=OP.add)
            # incremental n2 stats for this feature tile
            rbf = sqp.tile([128, SQ], BF16, name="rbf", tag="rbf", bufs=2)
            nc.gpsimd.tensor_copy(rbf[:], fout[m][:])
            sq_t = sqp.tile([128, SQ], BF16, name="sq", tag="sq", bufs=2)
            nc.scalar.square(sq_t[:], rbf[:])
            st_, sp_ = (m == 0), (m == KD - 1)
            nc.tensor.matmul(n2sp[:, 0, :], ones_bf[:, 0:128], rbf[:],
                             start=st_, stop=sp_)
            nc.tensor.matmul(n2sp[:, 1, :], ones_bf[:, 0:128], sq_t[:],
                             start=st_, stop=sp_)
        return n2sp

    def emit_n2(qc, n2sp):
        mu, rst = ln_finish(n2sp)

        def w_n2(kt, t1):
            stg = stgp.tile([128, SQ], F16, name="stg", tag="stg", bufs=2)
            nc.scalar.activation(stg[:], t1[:], AF.Identity,
                                 bias=ct["n2b"][:, kt:kt + 1],
                                 scale=ct["n2g"][:, kt:kt + 1])
            nc.sync.dma_start(io["outT"][kt * 128:(kt + 1) * 128, qs(qc)],
                              stg[:])

        ln_norm_simple(lambda kt: fout[kt][:], mu, rst, w_n2)

    n2sp0 = ffn_rest(0, f1_qc0)
    emit_n2(0, n2sp0)
    f1_qc1 = ffn_w1(1)
    n2sp1 = ffn_rest(1, f1_qc1)
    emit_n2(1, n2sp1)

    f2p.release()
    f1p.release()
    fop.release()
    wf.release()
    xbfp.release()
    xpp.release()
    rp.release()
    wd.release()
    stgp.release()
    smallp.release()
    sqp.release()
    vecp.release()
    tmpp.release()
    consts.release()
    psum.release()


# ------------------------------------------------------------------
# host side
# ------------------------------------------------------------------

def _shuffle_w(w):
    """[K, E] -> [128, (E//128)*K] bf16 so that slice [:, m*K:(m+1)*K]
    viewed as [128, K//128, 128] gives lhsT tiles w[kt*128+p, m*128+c]."""
    K, E = w.shape
    r = np.asarray(w).reshape(K // 128, 128, E // 128, 128).transpose(1, 2, 0, 3)
    return np.ascontiguousarray(r.reshape(128, (E // 128) * K)).astype(
        ml_dtypes.bfloat16)


def _ptable(b):
    """[E] -> [128, E//128] per-partition scalar table."""
    return np.ascontiguousarray(np.asarray(b, np.float32).reshape(-1, 128).T)


def _declare_io(nc, cfg, consts=None):
    """xT/outT are per-call I/O; every weight tensor is embedded in the
    NEFF as a Const DRAM tensor (DMA'd to HBM once at model-load time) so
    repeated executions only ship the activation tensors."""
    S, D, F, KD, KF = cfg.S, cfg.D, cfg.F, cfg.KD, cfg.KF
    io = {}

    def inp(name, shape, dt):
        if consts is not None and name in consts:
            arr = np.ascontiguousarray(consts[name])
            assert list(arr.shape) == list(shape), (name, arr.shape, shape)
            io[name] = nc.inline_tensor(arr, name=name).ap()
        else:
            io[name] = nc.dram_tensor(name, shape, dt, kind="ExternalInput").ap()

    inp("xT", [D, S], F16)
    inp("wqs", [128, KD * D], BF16)
    inp("wks", [128, KD * D], BF16)
    inp("wv", [D, D], BF16)
    inp("wos", [128, KD * D], BF16)
    inp("w1s", [128, KF * D], BF16)
    inp("w2s", [128, KF * F], BF16)
    inp("wgs", [128, KD * D], BF16)
    inp("w3s", [128, KD * F], BF16)
    inp("cpk", [128, 16 * KD + 2 * KF], F32)
    inp("bvr", [1, D], BF16)
    io["outT"] = nc.dram_tensor("outT", [D, S], F16, kind="ExternalOutput").ap()
    return io


def build_shared_inputs(inputs, cfg):
    """Everything except xT (identical across cores)."""
    f32 = np.float32
    g = {k: np.asarray(v) for k, v in inputs.items()}
    ptabs = [
        _ptable(g["bq"]), _ptable(g["bk"]),
        _ptable(np.asarray(g["bo"], f32) + 0.3 * np.asarray(g["conv_b"], f32)),
        _ptable(0.3 * np.asarray(g["conv_w"], f32)[:, 0]),
        _ptable(0.3 * np.asarray(g["conv_w"], f32)[:, 1]),
        _ptable(0.3 * np.asarray(g["conv_w"], f32)[:, 2]),
        _ptable(g["b3"]), _ptable(g["bg"]),
        _ptable(g["lna_g"]), _ptable(g["lna_b"]),
        _ptable(g["n1_g"]), _ptable(g["n1_b"]),
        _ptable(g["n2_g"]), _ptable(g["n2_b"]),
        _ptable(np.asarray(g["lna_b"], f32)
                / np.where(np.abs(np.asarray(g["lna_g"], f32)) < 1e-20,
                           1.0, np.asarray(g["lna_g"], f32))),
        _ptable(np.asarray(g["n1_b"], f32)
                / np.where(np.abs(np.asarray(g["n1_g"], f32)) < 1e-20,
                           1.0, np.asarray(g["n1_g"], f32))),
        _ptable(g["b1"]), _ptable(g["b2"]),
    ]
    sh = {
        "wqs": _shuffle_w(g["Wq"]), "wks": _shuffle_w(g["Wk"]),
        "wv": np.ascontiguousarray(g["Wv"]).astype(ml_dtypes.bfloat16),
        "wos": _shuffle_w(g["Wo"]), "w1s": _shuffle_w(g["W1"]),
        "w2s": _shuffle_w(g["W2"]), "w3s": _shuffle_w(g["W3"]),
        "wgs": _shuffle_w(g["Wg"]),
        "cpk": np.ascontiguousarray(np.concatenate(ptabs, axis=1)),
        "bvr": np.ascontiguousarray(
            np.asarray(g["bv"], f32).reshape(1, cfg.D)).astype(
                ml_dtypes.bfloat16),
    }
    return sh


_CACHE = {}


def _weights_fingerprint(inputs):
    """Content hash of every non-x input. The NEFF embeds the weights, so
    a changed weight set must rebuild (and recompile) the kernel."""
    import hashlib
    h = hashlib.blake2b(digest_size=16)
    for k in sorted(inputs):
        if k == "x":
            continue
        a = np.ascontiguousarray(inputs[k])
        h.update(k.encode())
        h.update(str(a.shape).encode())
        h.update(str(a.dtype).encode())
        h.update(a.tobytes())
    return h.hexdigest()


def _get_nc(inputs):
    key = _weights_fingerprint(inputs)
    ent = _CACHE.get("nc")
    if ent is None or ent[0] != key:
        shared = build_shared_inputs(inputs, FULL)
        nc = bass.Bass("TRN2", target_bir_lowering=False, debug=False)
        io = _declare_io(nc, FULL, consts=shared)
        with _TC(nc) as tc:
            emit(tc, FULL, io)
        # partition_id is declared by Bass but never referenced by this
        # pure data-parallel kernel; strip the allocation so executions
        # don't bind (and ship) a dead per-core buffer.
        if nc.partition_id_tensor is not None:
            pname = nc.partition_id_tensor.name
            referenced = any(
                pname in str(ap)
                for fn in nc.m.functions for bb in fn.blocks
                for inst in bb.instructions
                for ap in list(inst.ins or []) + list(inst.outs or [])
            )
            if not referenced:
                allocs = nc.m.functions[0].allocations
                for i in range(len(allocs) - 1, -1, -1):
                    a = allocs[i]
                    if (isinstance(a, mybir.MemoryLocationSet)
                            and a.kind == "ExternalInput"
                            and a.memorylocations[0].name == pname):
                        allocs.pop(i)
                nc.partition_id_tensor = None
        _CACHE["nc"] = (key, nc)
    return _CACHE["nc"][1]


def _get_exec(inputs):
    """Persistent jitted executor: one bass_exec custom call shard_mapped
    over the 8 cores.  Rebuilt only when the weight set changes."""
    import jax
    from jax.sharding import Mesh, PartitionSpec
    from jax.experimental.shard_map import shard_map
    from concourse import bass2jax

    key = _weights_fingerprint(inputs)
    ent = _CACHE.get("exec")
    if ent is not None and ent[0] == key:
        return ent[1]

    bass2jax.install_neuronx_cc_hook()
    nc = _get_nc(inputs)
    pid = nc.partition_id_tensor.name if nc.partition_id_tensor else None
    in_names, out_names, out_avals = [], [], []
    for alloc in nc.m.functions[0].allocations:
        if not isinstance(alloc, mybir.MemoryLocationSet):
            continue
        name = alloc.memorylocations[0].name
        if alloc.kind == "ExternalInput":
            if name != pid:
                in_names.append(name)
        elif alloc.kind == "ExternalOutput":
            out_names.append(name)
            out_avals.append(jax.core.ShapedArray(
                tuple(alloc.tensor_shape), mybir.dt.np(alloc.dtype)))
    all_names = in_names + ([pid] if pid is not None else [])
    assert in_names == ["xT"] and out_names == ["outT"]

    def _body(*args):
        operands = list(args)
        if pid is not None:
            operands.append(bass2jax.partition_id_tensor())
        return tuple(bass2jax._bass_exec_p.bind(
            *operands, out_avals=tuple(out_avals), in_names=tuple(all_names),
            out_names=tuple(out_names), lowering_input_output_aliases=(),
            sim_require_finite=True, sim_require_nnan=True, nc=nc))

    mesh = Mesh(np.asarray(jax.devices()[:N_CORES]), ("core",))
    sharded = jax.jit(
        shard_map(_body, mesh=mesh, in_specs=(PartitionSpec("core"),),
                  out_specs=(PartitionSpec("core"),), check_rep=False),
        keep_unused=True)
    _CACHE["exec"] = (key, sharded)
    return sharded


def kernel(**inputs):
    import jax

    cfg = FULL
    sharded = _get_exec(inputs)
    x = np.asarray(inputs["x"], dtype=np.float32)
    B = x.shape[0]
    assert B == N_CORES
    xc = np.concatenate([x[b].T.astype(np.float16, order="C")
                         for b in range(B)], axis=0)
    out = sharded(jax.device_put(xc))
    o0 = np.asarray(out[0]).astype(np.float32).reshape(B, cfg.D, cfg.S)
    return np.ascontiguousarray(o0.transpose(0, 2, 1))

